# revision 12
# baseline (speedup 1.0000x reference)
"""Trainium2 Bass kernel for the sparse-attention block (full attn + window attn + MLP).

v2 design (per core, data-parallel over sq_len: 16 frames/core):
  - Residual stream SBUF-RESIDENT: x2_sb [128, DC, 4096] bf16, feature-major.
    Stage 1 updates it in place (frame order); stage 2 gathers window-permuted
    views on-chip (no scatter DMAs, no DRAM roundtrip between attentions).
  - All matmul operands bf16 (f32 PSUM accumulate); LN gamma/beta folded into
    the weights on the host, so on-device LN is only (x - m) * r via two
    broadcast matmuls (R, M) + fused vector ops.
  - Attention: scores [s, t] via quadrant-packed bf16 matmuls; exp on ACT;
    denominators via ones-matmuls; att@v packs head pairs into one PSUM bank
    through output-partition offsets (no zero-padded v).
  - Software-pipelined emission: LN of group k+1 is emitted before the
    attention/MLP core of group k so TensorE never starves.
"""

import math
import numpy as np

D = 768
NH = 12
HD = 64
T = 256          # tokens per attention group
NF = 16          # frames (or windows) per core
NT = NF * T      # tokens per core
NCORE = 8
DC = 6           # feature chunks (768/128)
FH = 3072        # MLP hidden
FHC = 24         # hidden chunks
EPS = 1e-5

_CACHE = {}


def _build(with_mask: bool, iters: int = 1):
    import contextlib
    from concourse import bacc
    import concourse.mybir as mybir
    import concourse.tile as tile

    f32 = mybir.dt.float32
    bf16 = mybir.dt.bfloat16
    fp8 = mybir.dt.float8e4
    DR = mybir.MatmulPerfMode.DoubleRow
    AF = mybir.ActivationFunctionType
    OP = mybir.AluOpType

    nc = bacc.Bacc("TRN2", target_bir_lowering=False, debug=False, num_devices=NCORE)

    xt = nc.dram_tensor("xt", [8, 128, DC, 512], bf16, kind="ExternalInput")
    # per stage: q, k, v, o with LN gamma folded in (rows scaled by g)
    wad = nc.dram_tensor("wad", [2, 128, 4, DC, D], bf16, kind="ExternalInput")
    w1d = nc.dram_tensor("w1d", [128, DC, FH], bf16, kind="ExternalInput")
    w2d = nc.dram_tensor("w2d", [128, FHC, D], bf16, kind="ExternalInput")
    # biasd cols: 0:6 qb1', 6:12 kb1', 12:18 qb2', 18:24 kb2', 24:30 c1',
    #             30:36 c2', 36:42 f2b, 42:66 f1b'
    biasd = nc.dram_tensor("biasd", [128, 66], f32, kind="ExternalInput")
    # cvecd cols: 0 ones(stats); 1:3 = [1,0]; 3:5 = [0,1]
    cvecd = nc.dram_tensor("cvecd", [128, 8], bf16, kind="ExternalInput")
    # cvec8d cols 0:4 = [1,0,1,0] (hh0 DR sel); 4:8 = [0,1,0,1] (hh1)
    cvec8d = nc.dram_tensor("cvec8d", [128, 8], fp8, kind="ExternalInput")
    # rowd: [:,0:128] bcast1 (row0 ones); [:,128:256] sel2; [:,256:768] zeros
    rowd = nc.dram_tensor("rowd", [2, 768], bf16, kind="ExternalInput")
    if with_mask:
        maskd = nc.dram_tensor("maskd", [128, 2, T], f32, kind="ExternalInput")

    m2 = nc.dram_tensor("m2", [8, 128, DC, 512], bf16)          # window order
    ot = nc.dram_tensor("ot", [8, 128, DC, 512], f32, kind="ExternalOutput")

    with tile.TileContext(nc) as tc:
        est = contextlib.ExitStack()
        with est:
            est.enter_context(nc.allow_low_precision(
                reason="bf16 matmuls with f32 PSUM; rel-err budget 2e-2 measured"))
            cpool = est.enter_context(tc.tile_pool(name="consts", bufs=1))
            spool = est.enter_context(tc.tile_pool(name="small", bufs=2))
            hpool = est.enter_context(tc.tile_pool(name="heads", bufs=4))

            # ---------------- constants ----------------
            biasp = cpool.tile([128, 66], f32, tag="biasp")
            nc.sync.dma_start(biasp[:], biasd[:])
            cvec = cpool.tile([128, 8], bf16, tag="cvec")
            nc.sync.dma_start(cvec[:], cvecd[:])
            cvec8 = cpool.tile([128, 8], fp8, tag="cvec8")
            nc.sync.dma_start(cvec8[:], cvec8d[:])
            rowc = cpool.tile([2, 768], bf16, tag="rowc")
            nc.sync.dma_start(rowc[:], rowd[:])
            if with_mask:
                mask_sb = cpool.tile([128, 2, T], f32, tag="mask_sb")
                nc.sync.dma_start(mask_sb[:], maskd[:])

            ones_c = cvec[:, 0:1]
            bcast1 = rowc[:, 0:128]
            sel2 = rowc[:, 128:256]

            # persistent row-vector scratch (rows 1 stay zero forever)
            statrs = [cpool.tile([2, 512], bf16, tag=f"statr{i}", name=f"statr{i}")
                      for i in range(2)]
            mrzs = [cpool.tile([2, 512], bf16, tag=f"mrz{i}", name=f"mrz{i}")
                    for i in range(2)]
            for t_ in statrs + mrzs:
                nc.vector.tensor_copy(t_[:], rowc[:, 256:768])

            def bias_col(idx, n=1):
                return biasp[:, idx:idx + n]

            def body(iv=None):
                st2 = contextlib.ExitStack()
                with st2:
                    x2pool = st2.enter_context(tc.tile_pool(name="resid", bufs=1))
                    wpool = st2.enter_context(tc.tile_pool(name="wts", bufs=6))
                    apool = st2.enter_context(tc.tile_pool(name="acts", bufs=2))
                    opool = st2.enter_context(tc.tile_pool(name="outs", bufs=2))
                    pp = st2.enter_context(tc.tile_pool(name="psA", bufs=8, space="PSUM"))

                    def psum():
                        return pp.tile([128, 512], f32, tag="ps", name="ps")

                    x2 = x2pool.tile([128, DC, 4096], bf16, tag="x2", name="x2")
                    for p in range(8):
                        nc.sync.dma_start(x2[:, :, 512 * p:512 * (p + 1)], xt.ap()[p])

                    wt = {}
                    for s in range(2):
                        for mi, mn in enumerate("qkvo"):
                            w_ = wpool.tile([128, DC, D], bf16, tag="w", name=f"w{mn}{s}")
                            nc.gpsimd.dma_start(w_[:], wad.ap()[s, :, mi])
                            wt[(s, mn)] = w_

                    # ---------------- layernorm over 512 tokens (split emission) ----------------
                    def ln_stats(xviews, pidx, psum_fn):
                        """Stats matmuls + row chain -> statr/mrz rows. Releases PSUM fast."""
                        statr, mrz = statrs[pidx % 2], mrzs[pidx % 2]
                        st1, st2_ = psum_fn(), psum_fn()
                        for c in range(DC):
                            x2t = spool.tile([128, 512], bf16, tag="x2t", name="x2t", bufs=3)
                            nc.scalar.activation(x2t[:], xviews[c], AF.Square,
                                                 bias=0.0, scale=1.0)
                            nc.tensor.matmul(st1[0:1, :], ones_c, xviews[c],
                                             start=(c == 0), stop=(c == DC - 1))
                            nc.tensor.matmul(st2_[0:1, :], ones_c, x2t[:],
                                             start=(c == 0), stop=(c == DC - 1))
                        t_m = spool.tile([2, 512], f32, tag="vec", name="t_m", bufs=4)
                        t_e = spool.tile([2, 512], f32, tag="vec", name="t_e", bufs=4)
                        # m_neg = -Sx/D ; E2 = Sx2/D ; var = E2 - m^2 ; r = 1/sqrt(var+eps)
                        nc.vector.tensor_scalar_mul(t_m[0:1, :], st1[0:1, :], -1.0 / D)
                        nc.vector.tensor_scalar_mul(t_e[0:1, :], st2_[0:1, :], 1.0 / D)
                        nc.vector.tensor_tensor(mrz[0:1, :], t_m[0:1, :], t_m[0:1, :], OP.mult)
                        nc.vector.tensor_tensor(t_e[0:1, :], t_e[0:1, :], mrz[0:1, :], OP.subtract)
                        nc.vector.tensor_scalar_add(t_e[0:1, :], t_e[0:1, :], EPS)
                        nc.scalar.activation(t_e[0:1, :], t_e[0:1, :], AF.Sqrt,
                                             bias=0.0, scale=1.0)
                        nc.vector.reciprocal(statr[0:1, :], t_e[0:1, :])
                        nc.vector.tensor_tensor(mrz[0:1, :], t_m[0:1, :],
                                                statr[0:1, :], OP.mult)

                    def ln_finish(xviews, a_out, pidx, psum_fn):
                        """Broadcast R/M + normalize into a_out."""
                        statr, mrz = statrs[pidx % 2], mrzs[pidx % 2]
                        rps, mps = psum_fn(), psum_fn()
                        nc.tensor.matmul(rps[:, :], bcast1, statr[:, :], start=True, stop=True)
                        nc.tensor.matmul(mps[:, :], bcast1, mrz[:, :], start=True, stop=True)
                        rsb = spool.tile([128, 512], bf16, tag="rmsb", name="rsb", bufs=4)
                        msb = spool.tile([128, 512], bf16, tag="rmsb", name="msb", bufs=4)
                        nc.scalar.copy(rsb[:], rps[:, :])
                        nc.scalar.copy(msb[:], mps[:, :])
                        for c in range(DC):
                            tmp = spool.tile([128, 512], bf16, tag="x2t", name="lntmp", bufs=3)
                            nc.vector.tensor_tensor(tmp[:], xviews[c], rsb[:], OP.mult)
                            nc.vector.tensor_tensor(a_out[:, c, :], tmp[:], msb[:], OP.add)

                    # ---------------- attention core for one 512-token pair ----------------
                    def attn_qkv(stage, a1):
                        s = stage - 1
                        wq, wk, wv, wo = (wt[(s, m)] for m in "qkvo")
                        qb = bias_col(0 if stage == 1 else 12, DC)
                        kb = bias_col(6 if stage == 1 else 18, DC)

                        qT = apool.tile([128, DC, 512], bf16, tag="qT", name="qT", bufs=1)
                        kT = apool.tile([128, DC, 512], bf16, tag="kT", name="kT", bufs=1)
                        for oc in range(DC):
                            ps = psum()
                            for c in range(DC):
                                nc.tensor.matmul(ps[:, :], wq[:, c, 128 * oc:128 * (oc + 1)],
                                                 a1[:, c, :], start=(c == 0), stop=(c == DC - 1))
                            nc.scalar.activation(qT[:, oc, :], ps[:, :], AF.Identity,
                                                 bias=qb[:, oc:oc + 1], scale=1.0)
                            ps = psum()
                            for c in range(DC):
                                nc.tensor.matmul(ps[:, :], wk[:, c, 128 * oc:128 * (oc + 1)],
                                                 a1[:, c, :], start=(c == 0), stop=(c == DC - 1))
                            nc.vector.tensor_scalar_add(kT[:, oc, :], ps[:, :], kb[:, oc:oc + 1])
                        # v in [s, d] orientation: vv[:, sc, head, 0:64]
                        vv = apool.tile([128, 4, NH, HD], fp8, tag="vv", name="vv", bufs=2)
                        for sc in range(4):
                            for half in range(2):
                                ps = psum()
                                for c in range(DC):
                                    nc.tensor.matmul(ps[:, 0:384],
                                                     a1[:, c, 128 * sc:128 * (sc + 1)],
                                                     wv[:, c, 384 * half:384 * (half + 1)],
                                                     start=(c == 0), stop=(c == DC - 1))
                                nc.vector.tensor_copy(
                                    vv[:, sc, 6 * half:6 * (half + 1), :].rearrange(
                                        "p h d -> p (h d)"),
                                    ps[:, 0:384])
                        return qT, kT, vv

                    def attn_rest(stage, qkv, xviews, out_writer):
                        s = stage - 1
                        qT, kT, vv = qkv
                        wo = wt[(s, "o")]
                        cb = bias_col(24 if stage == 1 else 30, DC)
                        yT = apool.tile([128, DC, 512], bf16, tag="yT", name="yT", bufs=2)
                        for f in range(2):
                            toff = 256 * f
                            for hp in range(6):
                                sps = [psum(), psum()]
                                for sc2 in range(2):
                                    for hh in range(2):
                                        nc.tensor.matmul(
                                            sps[hh][:, 256 * sc2:256 * (sc2 + 1)],
                                            kT[64 * hh:64 * (hh + 1), hp,
                                               toff + 128 * sc2:toff + 128 * (sc2 + 1)],
                                            qT[64 * hh:64 * (hh + 1), hp, toff:toff + 256],
                                            start=(sc2 == 0), stop=(sc2 == 1),
                                            tile_position=(64 * hh, 0))
                                att = [None, None]
                                for hh in range(2):
                                    if with_mask and stage == 1:
                                        nc.vector.tensor_tensor(
                                            sps[hh][:, :], sps[hh][:, :],
                                            mask_sb[:].rearrange("p s t -> p (s t)"), OP.add)
                                    att[hh] = hpool.tile([128, 2, T], fp8, tag="att", name="att")
                                    nc.scalar.activation(
                                        att[hh][:].rearrange("p s t -> p (s t)"), sps[hh][:, :],
                                        AF.Exp, bias=0.0, scale=1.0 / math.sqrt(HD))
                                pd = psum()
                                for hh in range(2):
                                    nc.tensor.matmul(
                                        pd[0:2, 0:T],
                                        cvec8[:, 4 * hh:4 * hh + 4].rearrange(
                                            "p (i m) -> p i m", i=2),
                                        att[hh][:], perf_mode=DR,
                                        start=(hh == 0), stop=(hh == 1))
                                yb = psum()
                                for hh in range(2):
                                    nc.tensor.matmul(
                                        yb[64 * hh:64 * (hh + 1), 0:T],
                                        vv[:, 2 * f:2 * f + 2, 2 * hp + hh, :],
                                        att[hh][:], perf_mode=DR,
                                        start=True, stop=True)
                                pdr = spool.tile([2, 512], bf16, tag="pdr", name="pdr", bufs=4)
                                nc.vector.reciprocal(pdr[0:2, 0:T], pd[0:2, 0:T])
                                nc.tensor.matmul(pd[:, T:2 * T], sel2, pdr[0:2, 0:T],
                                                 start=True, stop=True)
                                r2sb = spool.tile([128, T], bf16, tag="r2sb", name="r2sb", bufs=4)
                                nc.vector.tensor_copy(r2sb[:], pd[:, T:2 * T])
                                nc.vector.tensor_tensor(yT[:, hp, toff:toff + 256],
                                                        yb[:, 0:T], r2sb[:], OP.mult)

                        for ec in range(DC):
                            ps = psum()
                            for hc in range(DC):
                                nc.tensor.matmul(ps[:, :], wo[:, hc, 128 * ec:128 * (ec + 1)],
                                                 yT[:, hc, :], start=(hc == 0), stop=(hc == DC - 1))
                            out_writer(ec, ps, cb, xviews)

                    # ================= stage 1 (frame order, in-place on x2) =================
                    def x2views(p):
                        return [x2[:, c, 512 * p:512 * (p + 1)] for c in range(DC)]

                    def s1_writer_for(p):
                        def wr(ec, ps, cb, xviews):
                            nc.vector.scalar_tensor_tensor(
                                x2[:, ec, 512 * p:512 * (p + 1)], ps[:, :],
                                cb[:, ec:ec + 1], xviews[ec], OP.add, OP.add)
                        return wr

                    a1s = {}
                    for p in range(9):
                        if p < 8:
                            ln_stats(x2views(p), p, psum)
                        if p >= 1:
                            qkv = attn_qkv(1, a1s.pop(p - 1))
                        if p < 8:
                            a1 = apool.tile([128, DC, 512], bf16, tag="a1", name="a1", bufs=2)
                            ln_finish(x2views(p), a1, p, psum)
                            a1s[p] = a1
                        if p >= 1:
                            attn_rest(1, qkv, x2views(p - 1), s1_writer_for(p - 1))

                    # ================= stage 2 (window order, gather from x2) =================
                    # x2 token idx = f*256 + (4hi+hh)*16 + 4wi+ww ; window w=(hi,wi)
                    x2w = x2[:].rearrange("p c (f hi hh wi ww) -> p c hi wi f hh ww",
                                          f=16, hi=4, hh=4, wi=4)

                    def gather(wp):
                        xs2 = opool.tile([128, DC, 512], bf16, tag="xs", name="xs2", bufs=2)
                        for c in range(DC):
                            for j in range(2):
                                w = 2 * wp + j
                                src = x2w[:, c, w // 4, w % 4]  # [128, 16, 4, 4]
                                dst = xs2[:, c, 256 * j:256 * (j + 1)]
                                nc.scalar.copy(
                                    dst.rearrange("p (f hh ww) -> p f hh ww", f=16, hh=4),
                                    src)
                        return xs2

                    def s2_writer_for(oo2):
                        def wr(ec, ps, cb, xviews):
                            nc.vector.scalar_tensor_tensor(
                                oo2[:, ec, :], ps[:, :], cb[:, ec:ec + 1],
                                xviews[ec], OP.add, OP.add)
                        return wr

                    prev = None
                    for wp in range(9):
                        if wp < 8:
                            xs2 = gather(wp)
                        if wp >= 1:
                            qkv = attn_qkv(2, prev[1])
                        if wp < 8:
                            ln_stats([xs2[:, c, :] for c in range(DC)], wp, psum)
                            a1 = apool.tile([128, DC, 512], bf16, tag="a1", name="a1b", bufs=2)
                            ln_finish([xs2[:, c, :] for c in range(DC)], a1, wp, psum)
                        if wp >= 1:
                            p_ = wp - 1
                            oo2 = opool.tile([128, DC, 512], bf16, tag="oo2", name="oo2", bufs=2)
                            attn_rest(2, qkv, [prev[0][:, c, :] for c in range(DC)],
                                      s2_writer_for(oo2))
                            nc.sync.dma_start(m2.ap()[p_], oo2[:])
                        if wp < 8:
                            prev = (xs2, a1)

                # ================= MLP (window order slabs from m2) =================
                st3 = contextlib.ExitStack()
                with st3:
                    wmpool = st3.enter_context(tc.tile_pool(name="wtsm", bufs=2))
                    mpool = st3.enter_context(tc.tile_pool(name="mact", bufs=2))
                    gpool = st3.enter_context(tc.tile_pool(name="gm", bufs=1))
                    ppB = st3.enter_context(tc.tile_pool(name="psB", bufs=8, space="PSUM"))

                    def psumB():
                        return ppB.tile([128, 512], f32, tag="psb", name="psb")

                    w1 = wmpool.tile([128, DC, FH], bf16, tag="wm", name="w1")
                    nc.gpsimd.dma_start(w1[:], w1d.ap())
                    w2 = wmpool.tile([128, FHC, D], bf16, tag="wm", name="w2")
                    nc.gpsimd.dma_start(w2[:], w2d.ap())
                    f1b = bias_col(42, FHC)
                    f2b = bias_col(36, DC)

                    def ln512B(xviews, a_out, pidx):
                        statr, mrz = statrs[pidx % 2], mrzs[pidx % 2]
                        st1, st2_ = psumB(), psumB()
                        for c in range(DC):
                            x2t = spool.tile([128, 512], bf16, tag="x2t", name="x2tB", bufs=3)
                            nc.scalar.activation(x2t[:], xviews[c], AF.Square,
                                                 bias=0.0, scale=1.0)
                            nc.tensor.matmul(st1[0:1, :], ones_c, xviews[c],
                                             start=(c == 0), stop=(c == DC - 1))
                            nc.tensor.matmul(st2_[0:1, :], ones_c, x2t[:],
                                             start=(c == 0), stop=(c == DC - 1))
                        t_m = spool.tile([2, 512], f32, tag="vec", name="t_mB", bufs=4)
                        t_e = spool.tile([2, 512], f32, tag="vec", name="t_eB", bufs=4)
                        nc.vector.tensor_scalar_mul(t_m[0:1, :], st1[0:1, :], -1.0 / D)
                        nc.vector.tensor_scalar_mul(t_e[0:1, :], st2_[0:1, :], 1.0 / D)
                        nc.vector.tensor_tensor(mrz[0:1, :], t_m[0:1, :], t_m[0:1, :], OP.mult)
                        nc.vector.tensor_tensor(t_e[0:1, :], t_e[0:1, :], mrz[0:1, :], OP.subtract)
                        nc.vector.tensor_scalar_add(t_e[0:1, :], t_e[0:1, :], EPS)
                        nc.scalar.activation(t_e[0:1, :], t_e[0:1, :], AF.Sqrt,
                                             bias=0.0, scale=1.0)
                        nc.vector.reciprocal(statr[0:1, :], t_e[0:1, :])
                        nc.vector.tensor_tensor(mrz[0:1, :], t_m[0:1, :],
                                                statr[0:1, :], OP.mult)
                        rps, mps = psumB(), psumB()
                        nc.tensor.matmul(rps[:, :], bcast1, statr[:, :], start=True, stop=True)
                        nc.tensor.matmul(mps[:, :], bcast1, mrz[:, :], start=True, stop=True)
                        rsb = spool.tile([128, 512], bf16, tag="rmsb", name="rsbB", bufs=4)
                        msb = spool.tile([128, 512], bf16, tag="rmsb", name="msbB", bufs=4)
                        nc.scalar.copy(rsb[:], rps[:, :])
                        nc.scalar.copy(msb[:], mps[:, :])
                        for c in range(DC):
                            tmp = spool.tile([128, 512], bf16, tag="x2t", name="lntmpB", bufs=3)
                            nc.vector.tensor_tensor(tmp[:], xviews[c], rsb[:], OP.mult)
                            nc.vector.tensor_tensor(a_out[:, c, :], tmp[:], msb[:], OP.add)

                    def mlp_fc1(h3):
                        g1 = gpool.tile([128, FHC, 512], bf16, tag="g1", name="g1", bufs=2)
                        for oc in range(FHC):
                            ps1 = psumB()
                            for c in range(DC):
                                nc.tensor.matmul(ps1[:, :], w1[:, c, 128 * oc:128 * (oc + 1)],
                                                 h3[:, c, :], start=(c == 0), stop=(c == DC - 1))
                            nc.scalar.activation(g1[:, oc, :], ps1[:, :], AF.Gelu,
                                                 bias=f1b[:, oc:oc + 1], scale=1.0)
                        return g1

                    def mlp_fc2(xs3, g1, sl):
                        oo = mpool.tile([128, DC, 512], f32, tag="oo", name="oo", bufs=2)
                        for ec in range(DC):
                            zps = psumB()
                            for oc in range(FHC):
                                nc.tensor.matmul(zps[:, :], w2[:, oc, 128 * ec:128 * (ec + 1)],
                                                 g1[:, oc, :], start=(oc == 0), stop=(oc == FHC - 1))
                            nc.vector.scalar_tensor_tensor(
                                oo[:, ec, :], zps[:, :], f2b[:, ec:ec + 1],
                                xs3[:, ec, :], OP.add, OP.add)
                        nc.sync.dma_start(ot.ap()[sl], oo[:])

                    prevm = None
                    for sl in range(9):
                        if sl >= 1:
                            g1p = mlp_fc1(prevm[1])
                        if sl < 8:
                            xs3 = mpool.tile([128, DC, 512], bf16, tag="xs3", name="xs3", bufs=2)
                            nc.sync.dma_start(xs3[:], m2.ap()[sl])
                            ln_stats([xs3[:, c, :] for c in range(DC)], sl, psumB)
                        if sl >= 1:
                            mlp_fc2(prevm[0], g1p, sl - 1)
                        if sl < 8:
                            h3 = mpool.tile([128, DC, 512], bf16, tag="h3", name="h3", bufs=2)
                            ln_finish([xs3[:, c, :] for c in range(DC)], h3, sl, psumB)
                            prevm = (xs3, h3)

            if iters == 1:
                body()
            else:
                with tc.For_i(0, iters, 1) as iv:
                    body(iv)

    nc.compile()
    return nc


def _host_prep(inputs):
    """Build per-core input maps (bf16 weights with LN folding)."""
    import ml_dtypes
    f32 = np.float32
    bfl = ml_dtypes.bfloat16
    x = np.asarray(inputs["x"], f32)
    mask = np.asarray(inputs["mask"])
    with_mask = not bool((mask == 1).all())

    g = {i: np.asarray(inputs[f"ln{i}_g"], f32) for i in (1, 2, 3)}
    b = {i: np.asarray(inputs[f"ln{i}_b"], f32) for i in (1, 2, 3)}

    def wmat(key):
        return np.asarray(inputs[key], f32)

    def pack_w(w):
        # [in, out] -> [128, kc, out]
        kc = w.shape[0] // 128
        return np.ascontiguousarray(
            w.reshape(kc, 128, w.shape[1]).transpose(1, 0, 2)).astype(bfl)

    com = {}
    wa = np.zeros((2, 128, 4, DC, D), bfl)
    for s in (1, 2):
        gs, bs = g[s], b[s]
        for mi, mn in enumerate(("q", "k", "v", "o")):
            w = wmat(f"{mn}{s}_w")
            if mn != "o":
                w = gs[:, None] * w
            wa[s - 1, :, mi] = pack_w(w).reshape(128, DC, D)
    com["wad"] = wa
    com["w1d"] = pack_w(g[3][:, None] * wmat("fc1_w")).reshape(128, DC, FH)
    com["w2d"] = pack_w(wmat("fc2_w")).reshape(128, FHC, D)

    def chunks(vec, w):
        return np.asarray(vec, f32).reshape(w, 128).T

    biasp = np.zeros((128, 66), f32)
    qb1 = b[1] @ wmat("q1_w") + np.asarray(inputs["q1_b"], f32)
    kb1 = b[1] @ wmat("k1_w") + np.asarray(inputs["k1_b"], f32)
    qb2 = b[2] @ wmat("q2_w") + np.asarray(inputs["q2_b"], f32)
    kb2 = b[2] @ wmat("k2_w") + np.asarray(inputs["k2_b"], f32)
    vb1 = b[1] @ wmat("v1_w") + np.asarray(inputs["v1_b"], f32)
    vb2 = b[2] @ wmat("v2_w") + np.asarray(inputs["v2_b"], f32)
    c1 = vb1 @ wmat("o1_w") + np.asarray(inputs["o1_b"], f32)
    c2 = vb2 @ wmat("o2_w") + np.asarray(inputs["o2_b"], f32)
    f1b = b[3] @ wmat("fc1_w") + np.asarray(inputs["fc1_b"], f32)
    biasp[:, 0:6] = chunks(qb1, DC)
    biasp[:, 6:12] = chunks(kb1, DC)
    biasp[:, 12:18] = chunks(qb2, DC)
    biasp[:, 18:24] = chunks(kb2, DC)
    biasp[:, 24:30] = chunks(c1, DC)
    biasp[:, 30:36] = chunks(c2, DC)
    biasp[:, 36:42] = chunks(np.asarray(inputs["fc2_b"], f32), DC)
    biasp[:, 42:66] = chunks(f1b, FHC)
    com["biasd"] = biasp

    cvecd = np.zeros((128, 8), bfl)
    cvecd[:, 0] = 1
    cvecd[:, 1] = 1
    cvecd[:, 4] = 1
    com["cvecd"] = cvecd
    cvec8d = np.zeros((128, 8), ml_dtypes.float8_e4m3fn)
    cvec8d[:, 0] = 1
    cvec8d[:, 2] = 1
    cvec8d[:, 5] = 1
    cvec8d[:, 7] = 1
    com["cvec8d"] = cvec8d

    rowd = np.zeros((2, 768), bfl)
    rowd[0, 0:128] = 1                      # bcast1 row0
    rowd[0, 128:192] = 1                    # sel2 row0 -> partitions 0:64
    rowd[1, 192:256] = 1                    # sel2 row1 -> partitions 64:128
    com["rowd"] = rowd

    if with_mask:
        madd = (mask.reshape(T, T).astype(f32) - 1.0) * 1e9
        com["maskd"] = np.ascontiguousarray(
            madd.T.reshape(2, 128, T).transpose(1, 0, 2))

    in_maps = []
    for c in range(NCORE):
        shard = x[NF * c:NF * (c + 1)]                  # [16, 256, 768]
        xT = shard.reshape(NT, D).T                     # [768, 4096]
        m = dict(com)
        m["xt"] = np.ascontiguousarray(
            xT.reshape(DC, 128, 8, 512).transpose(2, 1, 0, 3)).astype(bfl)
        in_maps.append(m)
    return in_maps, with_mask


def _host_post(results, dtype):
    """results: list of 8 dicts with 'ot' [8, 128, DC, 512] f32, window order."""
    out = np.empty((NCORE * NF, T, D), dtype)
    for c, r in enumerate(results):
        o = np.asarray(r["ot"], np.float32)             # [8, 128, 6, 512]
        o = o.transpose(2, 1, 0, 3).reshape(D, NT)      # [768, 4096] wtoken order
        # wtoken = (hi, wi, f, hh, ww)
        o = o.reshape(D, 4, 4, NF, 4, 4)
        o = o.transpose(3, 1, 4, 2, 5, 0)               # (f, hi, hh, wi, ww, d)
        out[NF * c:NF * (c + 1)] = o.reshape(NF, T, D)
    return out


def kernel(**inputs) -> np.ndarray:
    from concourse.bass_utils import run_bass_kernel_spmd

    in_maps, with_mask = _host_prep(inputs)
    key = ("k", with_mask)
    if key not in _CACHE:
        _CACHE[key] = _build(with_mask)
    nc = _CACHE[key]
    res = run_bass_kernel_spmd(nc, in_maps, core_ids=list(range(NCORE)))
    return _host_post(res.results, np.asarray(inputs["x"]).dtype)


# revision 27
# speedup vs baseline: 3.6375x; 3.6375x over previous
"""Trainium2 Bass kernel for the sparse-attention block (full attn + window attn + MLP).

v2 design (per core, data-parallel over sq_len: 16 frames/core):
  - Residual stream SBUF-RESIDENT: x2_sb [128, DC, 4096] bf16, feature-major.
    Stage 1 updates it in place (frame order); stage 2 gathers window-permuted
    views on-chip (no scatter DMAs, no DRAM roundtrip between attentions).
  - All matmul operands bf16 (f32 PSUM accumulate); LN gamma/beta folded into
    the weights on the host, so on-device LN is only (x - m) * r via two
    broadcast matmuls (R, M) + fused vector ops.
  - Attention: scores [s, t] via quadrant-packed bf16 matmuls; exp on ACT;
    denominators via ones-matmuls; att@v packs head pairs into one PSUM bank
    through output-partition offsets (no zero-padded v).
  - Software-pipelined emission: LN of group k+1 is emitted before the
    attention/MLP core of group k so TensorE never starves.
"""

import math
import numpy as np

D = 768
NH = 12
HD = 64
T = 256          # tokens per attention group
NF = 16          # frames (or windows) per core
NT = NF * T      # tokens per core
NCORE = 8
DC = 6           # feature chunks (768/128)
FH = 3072        # MLP hidden
FHC = 24         # hidden chunks
EPS = 1e-5

_CACHE = {}


def _build(with_mask: bool, iters: int = 1):
    import contextlib
    from concourse import bacc
    import concourse.mybir as mybir
    import concourse.tile as tile

    f32 = mybir.dt.float32
    bf16 = mybir.dt.bfloat16
    fp8 = mybir.dt.float8e4
    DR = mybir.MatmulPerfMode.DoubleRow
    AF = mybir.ActivationFunctionType
    OP = mybir.AluOpType

    nc = bacc.Bacc("TRN2", target_bir_lowering=False, debug=False, num_devices=NCORE)

    xt = nc.dram_tensor("xt", [8, 128, DC, 512], bf16, kind="ExternalInput")
    # per stage: q, k, v, o with LN gamma folded in (rows scaled by g)
    wad = nc.dram_tensor("wad", [2, 128, 4, DC, D], bf16, kind="ExternalInput")
    w1d = nc.dram_tensor("w1d", [128, DC, FH], bf16, kind="ExternalInput")
    w2d = nc.dram_tensor("w2d", [128, FHC, D], bf16, kind="ExternalInput")
    # biasd cols: 0:6 qb1', 6:12 kb1', 12:18 qb2', 18:24 kb2', 24:30 c1',
    #             30:36 c2', 36:42 f2b, 42:66 f1b'
    biasd = nc.dram_tensor("biasd", [128, 66], f32, kind="ExternalInput")
    # cvecd cols: 0 ones(stats); 1:3 = [1,0]; 3:5 = [0,1]
    cvecd = nc.dram_tensor("cvecd", [128, 8], bf16, kind="ExternalInput")
    # cvec8d [128, 2, 16]: [:, i, 0:2] = [1,0] (hh0 DR sel); [:, i, 2:4] = [0,1]
    cvec8d = nc.dram_tensor("cvec8d", [128, 2, 16], fp8, kind="ExternalInput")
    # rowd: [:,0:128] bcast1 (row0 ones); [:,128:256] sel2; [:,256:768] zeros
    rowd = nc.dram_tensor("rowd", [2, 768], bf16, kind="ExternalInput")
    if with_mask:
        maskd = nc.dram_tensor("maskd", [128, 2, T], f32, kind="ExternalInput")

    m2 = nc.dram_tensor("m2", [8, 128, DC, 512], bf16)          # window order
    ot = nc.dram_tensor("ot", [8, 128, DC, 512], f32, kind="ExternalOutput")

    with tile.TileContext(nc) as tc:
        est = contextlib.ExitStack()
        with est:
            est.enter_context(nc.allow_low_precision(
                reason="bf16 matmuls with f32 PSUM; rel-err budget 2e-2 measured"))
            cpool = est.enter_context(tc.tile_pool(name="consts", bufs=1))
            spool = est.enter_context(tc.tile_pool(name="small", bufs=2))
            hpool = est.enter_context(tc.tile_pool(name="heads", bufs=4))

            # ---------------- constants ----------------
            cvec = cpool.tile([128, 8], bf16, tag="cvec")
            nc.sync.dma_start(cvec[:], cvecd[:])
            biasp = cpool.tile([128, 66], f32, tag="biasp")
            nc.sync.dma_start(biasp[:], biasd[:])
            cvec8 = cpool.tile([128, 2, 16], fp8, tag="cvec8")
            nc.sync.dma_start(cvec8[:], cvec8d[:])
            rowc = cpool.tile([2, 768], bf16, tag="rowc")
            nc.sync.dma_start(rowc[:], rowd[:])
            if with_mask:
                mask_sb = cpool.tile([128, 2, T], f32, tag="mask_sb")
                nc.sync.dma_start(mask_sb[:], maskd[:])

            ones_c = cvec[:, 0:1]
            bcast1 = rowc[:, 0:128]
            sel2 = rowc[:, 128:256]

            # persistent row-vector scratch (rows 1 stay zero forever)
            statrs = [cpool.tile([2, 512], bf16, tag=f"statr{i}", name=f"statr{i}")
                      for i in range(2)]
            mrzs = [cpool.tile([2, 512], bf16, tag=f"mrz{i}", name=f"mrz{i}")
                    for i in range(2)]
            for t_ in statrs + mrzs:
                nc.scalar.memzero(t_[:])

            def bias_col(idx, n=1):
                return biasp[:, idx:idx + n]

            def body(iv=None):
                st2 = contextlib.ExitStack()
                with st2:
                    x2pool = st2.enter_context(tc.tile_pool(name="resid", bufs=1))
                    wpool = st2.enter_context(tc.tile_pool(name="wts", bufs=6))
                    apool = st2.enter_context(tc.tile_pool(name="acts", bufs=2))
                    opool = st2.enter_context(tc.tile_pool(name="outs", bufs=2))
                    pp = st2.enter_context(tc.tile_pool(name="psA", bufs=8, space="PSUM"))

                    def psum():
                        return pp.tile([128, 512], f32, tag="ps", name="ps", bufs=4)

                    zero_vzp = [2]
                    x2 = x2pool.tile([128, DC, 4096], bf16, tag="x2", name="x2")
                    for p in range(8):
                        nc.sync.dma_start(x2[:, :, 512 * p:512 * (p + 1)], xt.ap()[p])

                    wt = {}
                    for s in range(2):
                        for mi, mn in enumerate("qkvo"):
                            w_ = wpool.tile([128, DC, D], bf16, tag="w", name=f"w{mn}{s}")
                            nc.gpsimd.dma_start(w_[:], wad.ap()[s, :, mi])
                            wt[(s, mn)] = w_

                    # ---------------- layernorm over 512 tokens (split emission) ----------------
                    def ln_stats(xviews, pidx, psum_fn, sq_act=False):
                        """Stats matmuls + row chain -> statr/mrz rows. Releases PSUM fast."""
                        statr, mrz = statrs[pidx % 2], mrzs[pidx % 2]
                        st1, st2_ = psum_fn(), psum_fn()
                        for c in range(DC):
                            x2t = spool.tile([128, 512], bf16, tag="x2t", name="x2t", bufs=3)
                            if sq_act:
                                nc.scalar.activation(x2t[:], xviews[c], AF.Square,
                                                     bias=0.0, scale=1.0)
                            else:
                                nc.vector.tensor_tensor(x2t[:], xviews[c], xviews[c], OP.mult)
                            nc.tensor.matmul(st1[0:1, :], ones_c, xviews[c],
                                             start=(c == 0), stop=(c == DC - 1))
                            nc.tensor.matmul(st2_[0:1, :], ones_c, x2t[:],
                                             start=(c == 0), stop=(c == DC - 1))
                        t_m = spool.tile([2, 512], f32, tag="vec", name="t_m", bufs=4)
                        t_e = spool.tile([2, 512], f32, tag="vec", name="t_e", bufs=4)
                        # m_neg = -Sx/D ; E2 = Sx2/D ; var = E2 - m^2
                        # r = 1/sqrt(var+eps) = exp(-0.5*log(var+eps)) -- stays in the
                        # exp/log ACT table set (no per-group table reload)
                        nc.vector.tensor_scalar_mul(t_m[0:1, :], st1[0:1, :], -1.0 / D)
                        nc.vector.tensor_scalar_mul(t_e[0:1, :], st2_[0:1, :], 1.0 / D)
                        nc.vector.tensor_tensor(mrz[0:1, :], t_m[0:1, :], t_m[0:1, :], OP.mult)
                        nc.vector.tensor_tensor(t_e[0:1, :], t_e[0:1, :], mrz[0:1, :], OP.subtract)
                        nc.vector.tensor_scalar_add(t_e[0:1, :], t_e[0:1, :], EPS)
                        nc.scalar.activation(t_e[0:1, :], t_e[0:1, :], AF.Ln,
                                             bias=0.0, scale=1.0)
                        nc.scalar.activation(statr[0:1, :], t_e[0:1, :], AF.Exp,
                                             bias=0.0, scale=-0.5)
                        nc.vector.tensor_tensor(mrz[0:1, :], t_m[0:1, :],
                                                statr[0:1, :], OP.mult)

                    def ln_finish(xviews, a_out, pidx, psum_fn):
                        """Broadcast R/M + normalize into a_out."""
                        statr, mrz = statrs[pidx % 2], mrzs[pidx % 2]
                        rps, mps = psum_fn(), psum_fn()
                        nc.tensor.matmul(rps[:, :], bcast1, statr[:, :], start=True, stop=True)
                        nc.tensor.matmul(mps[:, :], bcast1, mrz[:, :], start=True, stop=True)
                        rsb = spool.tile([128, 512], bf16, tag="rmsb", name="rsb", bufs=4)
                        msb = spool.tile([128, 512], bf16, tag="rmsb", name="msb", bufs=4)
                        nc.scalar.copy(rsb[:], rps[:, :])
                        nc.scalar.copy(msb[:], mps[:, :])
                        for c in range(DC):
                            tmp = spool.tile([128, 512], bf16, tag="x2t", name="lntmp", bufs=3)
                            nc.vector.tensor_tensor(tmp[:], xviews[c], rsb[:], OP.mult)
                            nc.vector.tensor_tensor(a_out[:, c, :], tmp[:], msb[:], OP.add)

                    # ---------------- attention core for one 512-token pair ----------------
                    def attn_qkv(stage, a1):
                        s = stage - 1
                        wq, wk, wv, wo = (wt[(s, m)] for m in "qkvo")
                        qb = bias_col(0 if stage == 1 else 12, DC)
                        kb = bias_col(6 if stage == 1 else 18, DC)

                        qT = apool.tile([128, DC, 512], bf16, tag="qT", name="qT", bufs=1)
                        kT = apool.tile([128, DC, 512], bf16, tag="kT", name="kT", bufs=1)
                        for oc in range(DC):
                            ps = psum()
                            for c in range(DC):
                                nc.tensor.matmul(ps[:, :], wq[:, c, 128 * oc:128 * (oc + 1)],
                                                 a1[:, c, :], start=(c == 0), stop=(c == DC - 1))
                            if stage == 1:
                                nc.scalar.activation(qT[:, oc, :], ps[:, :], AF.Identity,
                                                     bias=qb[:, oc:oc + 1], scale=1.0)
                            else:
                                nc.vector.tensor_scalar_add(qT[:, oc, :], ps[:, :],
                                                            qb[:, oc:oc + 1])
                            ps = psum()
                            for c in range(DC):
                                nc.tensor.matmul(ps[:, :], wk[:, c, 128 * oc:128 * (oc + 1)],
                                                 a1[:, c, :], start=(c == 0), stop=(c == DC - 1))
                            nc.vector.tensor_scalar_add(kT[:, oc, :], ps[:, :], kb[:, oc:oc + 1])
                        # v in [s, d] orientation, zero-padded per head pair:
                        # vzp[:, f, hp, hh, sc2, :]: hh=0 -> [v|0], hh=1 -> [0|v]
                        vzp = apool.tile([128, 2, 6, 2, 2, 128], fp8, tag="vv",
                                         name="vzp", bufs=2)
                        if zero_vzp[0] > 0:
                            zero_vzp[0] -= 1
                            nc.scalar.memzero(
                                vzp[:].rearrange("p a b c d e -> p (a b c d e)"))
                        for sc in range(4):
                            f, sc2 = sc // 2, sc % 2
                            for half in range(2):
                                ps = psum()
                                for c in range(DC):
                                    nc.tensor.matmul(ps[:, 0:384],
                                                     a1[:, c, 128 * sc:128 * (sc + 1)],
                                                     wv[:, c, 384 * half:384 * (half + 1)],
                                                     start=(c == 0), stop=(c == DC - 1))
                                psv = ps[:, 0:384].rearrange(
                                    "p (hp hh d) -> p hp hh d", hp=3, hh=2)
                                for hh in range(2):
                                    nc.vector.tensor_copy(
                                        vzp[:, f, 3 * half:3 * (half + 1), hh, sc2,
                                            64 * hh:64 * hh + 64],
                                        psv[:, :, hh, :])
                        return qT, kT, vzp

                    def attn_rest(stage, qkv, xviews, out_writer):
                        s = stage - 1
                        qT, kT, vzp = qkv
                        wo = wt[(s, "o")]
                        cb = bias_col(24 if stage == 1 else 30, DC)
                        yT = apool.tile([128, DC, 512], bf16, tag="yT", name="yT", bufs=2)
                        for f in range(2):
                            toff = 256 * f
                            for hp in range(6):
                                sps = pp.tile([128, 2, 512], f32, tag="ps2", name="sps", bufs=2)
                                for sc2 in range(2):
                                    for hh in range(2):
                                        nc.tensor.matmul(
                                            sps[:, hh, 256 * sc2:256 * (sc2 + 1)],
                                            kT[64 * hh:64 * (hh + 1), hp,
                                               toff + 128 * sc2:toff + 128 * (sc2 + 1)],
                                            qT[64 * hh:64 * (hh + 1), hp, toff:toff + 256],
                                            start=(sc2 == 0), stop=(sc2 == 1),
                                            tile_position=(64 * hh, 0))
                                if with_mask and stage == 1:
                                    for hh in range(2):
                                        nc.vector.tensor_tensor(
                                            sps[:, hh, :], sps[:, hh, :],
                                            mask_sb[:].rearrange("p s t -> p (s t)"), OP.add)
                                attb = hpool.tile([128, 2, 2, T], fp8, tag="att", name="attb")
                                nc.scalar.activation(
                                    attb[:].rearrange("p h s t -> p (h s t)"),
                                    sps[:].rearrange("p h x -> p (h x)"),
                                    AF.Exp, bias=0.0, scale=1.0 / math.sqrt(HD))
                                att = [attb[:, 0], attb[:, 1]]
                                pd = psum()
                                for hh in range(2):
                                    nc.tensor.matmul(
                                        pd[0:2, 0:T],
                                        cvec8[:, :, 2 * hh:2 * hh + 2],
                                        att[hh], perf_mode=DR,
                                        start=(hh == 0), stop=(hh == 1))
                                yb = psum()
                                for hh in range(2):
                                    nc.tensor.matmul(
                                        yb[:, 0:T],
                                        vzp[:, f, hp, hh],
                                        att[hh], perf_mode=DR,
                                        start=(hh == 0), stop=(hh == 1))
                                pdr = spool.tile([2, 512], bf16, tag="pdr", name="pdr", bufs=4)
                                nc.vector.reciprocal(pdr[0:2, 0:T], pd[0:2, 0:T])
                                nc.tensor.matmul(pd[:, T:2 * T], sel2, pdr[0:2, 0:T],
                                                 start=True, stop=True)
                                r2sb = spool.tile([128, T], bf16, tag="r2sb", name="r2sb", bufs=4)
                                nc.vector.tensor_copy(r2sb[:], pd[:, T:2 * T])
                                nc.vector.tensor_tensor(yT[:, hp, toff:toff + 256],
                                                        yb[:, 0:T], r2sb[:], OP.mult)

                        for ec in range(DC):
                            ps = psum()
                            for hc in range(DC):
                                nc.tensor.matmul(ps[:, :], wo[:, hc, 128 * ec:128 * (ec + 1)],
                                                 yT[:, hc, :], start=(hc == 0), stop=(hc == DC - 1))
                            out_writer(ec, ps, cb, xviews)

                    # ================= stage 1 (frame order, in-place on x2) =================
                    def x2views(p):
                        return [x2[:, c, 512 * p:512 * (p + 1)] for c in range(DC)]

                    def s1_writer_for(p):
                        def wr(ec, ps, cb, xviews):
                            nc.vector.scalar_tensor_tensor(
                                x2[:, ec, 512 * p:512 * (p + 1)], ps[:, :],
                                cb[:, ec:ec + 1], xviews[ec], OP.add, OP.add)
                        return wr

                    a1s = {}
                    for p in range(9):
                        if p < 8:
                            ln_stats(x2views(p), p, psum)
                        if p >= 1:
                            qkv = attn_qkv(1, a1s.pop(p - 1))
                        if p < 8:
                            a1 = apool.tile([128, DC, 512], bf16, tag="a1", name="a1", bufs=2)
                            ln_finish(x2views(p), a1, p, psum)
                            a1s[p] = a1
                        if p >= 1:
                            attn_rest(1, qkv, x2views(p - 1), s1_writer_for(p - 1))

                    # ================= stage 2 (window order, gather from x2) =================
                    # x2 token idx = f*256 + (4hi+hh)*16 + 4wi+ww ; window w=(hi,wi)
                    x2w = x2[:].rearrange("p c (f hi hh wi ww) -> p c hi wi f hh ww",
                                          f=16, hi=4, hh=4, wi=4)

                    def ln_stats_s2(wp, pidx):
                        """Stage-2 stats direct from x2 (strided views, pre-gather)."""
                        statr, mrz = statrs[pidx % 2], mrzs[pidx % 2]
                        st1, st2_ = psum(), psum()
                        for c in range(DC):
                            for j in range(2):
                                w = 2 * wp + j
                                src_ = x2w[:, c, w // 4, w % 4]   # [128, 16, 4, 4]
                                x2t = spool.tile([128, 256], bf16, tag="x2t2",
                                                 name="x2t2", bufs=3)
                                x2tv = x2t[:].rearrange("p (f hh ww) -> p f hh ww",
                                                        f=16, hh=4)
                                nc.vector.tensor_tensor(x2tv, src_, src_, OP.mult)
                                nc.tensor.matmul(st1[0:1, 256 * j:256 * (j + 1)],
                                                 ones_c, src_,
                                                 start=(c == 0), stop=(c == DC - 1))
                                nc.tensor.matmul(st2_[0:1, 256 * j:256 * (j + 1)],
                                                 ones_c, x2t[:],
                                                 start=(c == 0), stop=(c == DC - 1))
                        t_m = spool.tile([2, 512], f32, tag="vec", name="t_m2", bufs=4)
                        t_e = spool.tile([2, 512], f32, tag="vec", name="t_e2", bufs=4)
                        nc.vector.tensor_scalar_mul(t_m[0:1, :], st1[0:1, :], -1.0 / D)
                        nc.vector.tensor_scalar_mul(t_e[0:1, :], st2_[0:1, :], 1.0 / D)
                        nc.vector.tensor_tensor(mrz[0:1, :], t_m[0:1, :], t_m[0:1, :], OP.mult)
                        nc.vector.tensor_tensor(t_e[0:1, :], t_e[0:1, :], mrz[0:1, :], OP.subtract)
                        nc.vector.tensor_scalar_add(t_e[0:1, :], t_e[0:1, :], EPS)
                        nc.scalar.activation(t_e[0:1, :], t_e[0:1, :], AF.Ln,
                                             bias=0.0, scale=1.0)
                        nc.scalar.activation(statr[0:1, :], t_e[0:1, :], AF.Exp,
                                             bias=0.0, scale=-0.5)
                        nc.vector.tensor_tensor(mrz[0:1, :], t_m[0:1, :],
                                                statr[0:1, :], OP.mult)

                    def gather(wp):
                        xs2 = opool.tile([128, DC, 512], bf16, tag="xs", name="xs2", bufs=2)
                        for c in range(DC):
                            for j in range(2):
                                w = 2 * wp + j
                                src = x2w[:, c, w // 4, w % 4]  # [128, 16, 4, 4]
                                dst = xs2[:, c, 256 * j:256 * (j + 1)]
                                nc.scalar.copy(
                                    dst.rearrange("p (f hh ww) -> p f hh ww", f=16, hh=4),
                                    src)
                        return xs2

                    def s2_writer_for(oo2):
                        def wr(ec, ps, cb, xviews):
                            nc.vector.scalar_tensor_tensor(
                                oo2[:, ec, :], ps[:, :], cb[:, ec:ec + 1],
                                xviews[ec], OP.add, OP.add)
                        return wr

                    prev = None
                    for wp in range(9):
                        if wp < 8:
                            ln_stats_s2(wp, wp)
                            xs2 = gather(wp)
                        if wp >= 1:
                            qkv = attn_qkv(2, prev[1])
                        if wp < 8:
                            a1 = apool.tile([128, DC, 512], bf16, tag="a1", name="a1b", bufs=2)
                            ln_finish([xs2[:, c, :] for c in range(DC)], a1, wp, psum)
                        if wp >= 1:
                            p_ = wp - 1
                            oo2 = opool.tile([128, DC, 512], bf16, tag="oo2", name="oo2", bufs=2)
                            attn_rest(2, qkv, [prev[0][:, c, :] for c in range(DC)],
                                      s2_writer_for(oo2))
                            nc.sync.dma_start(m2.ap()[p_], oo2[:])
                        if wp < 8:
                            prev = (xs2, a1)

                # ================= MLP (window order slabs from m2) =================
                st3 = contextlib.ExitStack()
                with st3:
                    wmpool = st3.enter_context(tc.tile_pool(name="wtsm", bufs=2))
                    mpool = st3.enter_context(tc.tile_pool(name="mact", bufs=2))
                    gpool = st3.enter_context(tc.tile_pool(name="gm", bufs=1))
                    ppB = st3.enter_context(tc.tile_pool(name="psB", bufs=8, space="PSUM"))

                    def psumB():
                        return ppB.tile([128, 512], f32, tag="psb", name="psb")

                    w1 = wmpool.tile([128, DC, FH], bf16, tag="wm", name="w1")
                    nc.gpsimd.dma_start(w1[:], w1d.ap())
                    w2 = wmpool.tile([128, FHC, D], bf16, tag="wm", name="w2")
                    nc.gpsimd.dma_start(w2[:], w2d.ap())
                    f1b = bias_col(42, FHC)
                    f2b = bias_col(36, DC)

                    def ln512B(xviews, a_out, pidx):
                        statr, mrz = statrs[pidx % 2], mrzs[pidx % 2]
                        st1, st2_ = psumB(), psumB()
                        for c in range(DC):
                            x2t = spool.tile([128, 512], bf16, tag="x2t", name="x2tB", bufs=3)
                            nc.scalar.activation(x2t[:], xviews[c], AF.Square,
                                                 bias=0.0, scale=1.0)
                            nc.tensor.matmul(st1[0:1, :], ones_c, xviews[c],
                                             start=(c == 0), stop=(c == DC - 1))
                            nc.tensor.matmul(st2_[0:1, :], ones_c, x2t[:],
                                             start=(c == 0), stop=(c == DC - 1))
                        t_m = spool.tile([2, 512], f32, tag="vec", name="t_mB", bufs=4)
                        t_e = spool.tile([2, 512], f32, tag="vec", name="t_eB", bufs=4)
                        nc.vector.tensor_scalar_mul(t_m[0:1, :], st1[0:1, :], -1.0 / D)
                        nc.vector.tensor_scalar_mul(t_e[0:1, :], st2_[0:1, :], 1.0 / D)
                        nc.vector.tensor_tensor(mrz[0:1, :], t_m[0:1, :], t_m[0:1, :], OP.mult)
                        nc.vector.tensor_tensor(t_e[0:1, :], t_e[0:1, :], mrz[0:1, :], OP.subtract)
                        nc.vector.tensor_scalar_add(t_e[0:1, :], t_e[0:1, :], EPS)
                        nc.scalar.activation(t_e[0:1, :], t_e[0:1, :], AF.Sqrt,
                                             bias=0.0, scale=1.0)
                        nc.vector.reciprocal(statr[0:1, :], t_e[0:1, :])
                        nc.vector.tensor_tensor(mrz[0:1, :], t_m[0:1, :],
                                                statr[0:1, :], OP.mult)
                        rps, mps = psumB(), psumB()
                        nc.tensor.matmul(rps[:, :], bcast1, statr[:, :], start=True, stop=True)
                        nc.tensor.matmul(mps[:, :], bcast1, mrz[:, :], start=True, stop=True)
                        rsb = spool.tile([128, 512], bf16, tag="rmsb", name="rsbB", bufs=4)
                        msb = spool.tile([128, 512], bf16, tag="rmsb", name="msbB", bufs=4)
                        nc.scalar.copy(rsb[:], rps[:, :])
                        nc.scalar.copy(msb[:], mps[:, :])
                        for c in range(DC):
                            tmp = spool.tile([128, 512], bf16, tag="x2t", name="lntmpB", bufs=3)
                            nc.vector.tensor_tensor(tmp[:], xviews[c], rsb[:], OP.mult)
                            nc.vector.tensor_tensor(a_out[:, c, :], tmp[:], msb[:], OP.add)

                    def mlp_fc1(h3):
                        g1 = gpool.tile([128, FHC, 512], bf16, tag="g1", name="g1", bufs=2)
                        for oc in range(FHC):
                            ps1 = psumB()
                            for c in range(DC):
                                nc.tensor.matmul(ps1[:, :], w1[:, c, 128 * oc:128 * (oc + 1)],
                                                 h3[:, c, :], start=(c == 0), stop=(c == DC - 1))
                            nc.scalar.activation(g1[:, oc, :], ps1[:, :], AF.Gelu,
                                                 bias=f1b[:, oc:oc + 1], scale=1.0)
                        return g1

                    def mlp_fc2(xs3, g1, sl):
                        oo = mpool.tile([128, DC, 512], f32, tag="oo", name="oo", bufs=2)
                        for ec in range(DC):
                            zps = psumB()
                            for oc in range(FHC):
                                nc.tensor.matmul(zps[:, :], w2[:, oc, 128 * ec:128 * (ec + 1)],
                                                 g1[:, oc, :], start=(oc == 0), stop=(oc == FHC - 1))
                            nc.vector.scalar_tensor_tensor(
                                oo[:, ec, :], zps[:, :], f2b[:, ec:ec + 1],
                                xs3[:, ec, :], OP.add, OP.add)
                        nc.sync.dma_start(ot.ap()[sl], oo[:])

                    prevm = None
                    for sl in range(9):
                        if sl >= 1:
                            g1p = mlp_fc1(prevm[1])
                        if sl < 8:
                            xs3 = mpool.tile([128, DC, 512], bf16, tag="xs3", name="xs3", bufs=2)
                            nc.sync.dma_start(xs3[:], m2.ap()[sl])
                            ln_stats([xs3[:, c, :] for c in range(DC)], sl, psumB)
                        if sl >= 1:
                            mlp_fc2(prevm[0], g1p, sl - 1)
                        if sl < 8:
                            h3 = mpool.tile([128, DC, 512], bf16, tag="h3", name="h3", bufs=2)
                            ln_finish([xs3[:, c, :] for c in range(DC)], h3, sl, psumB)
                            prevm = (xs3, h3)

            if iters == 1:
                body()
            else:
                with tc.For_i(0, iters, 1) as iv:
                    body(iv)

    nc.compile()
    return nc


def _host_prep(inputs):
    """Build per-core input maps (bf16 weights with LN folding)."""
    import ml_dtypes
    f32 = np.float32
    bfl = ml_dtypes.bfloat16
    x = np.asarray(inputs["x"], f32)
    mask = np.asarray(inputs["mask"])
    with_mask = not bool((mask == 1).all())

    g = {i: np.asarray(inputs[f"ln{i}_g"], f32) for i in (1, 2, 3)}
    b = {i: np.asarray(inputs[f"ln{i}_b"], f32) for i in (1, 2, 3)}

    def wmat(key):
        return np.asarray(inputs[key], f32)

    def pack_w(w):
        # [in, out] -> [128, kc, out]
        kc = w.shape[0] // 128
        return np.ascontiguousarray(
            w.reshape(kc, 128, w.shape[1]).transpose(1, 0, 2)).astype(bfl)

    com = {}
    wa = np.zeros((2, 128, 4, DC, D), bfl)
    for s in (1, 2):
        gs, bs = g[s], b[s]
        for mi, mn in enumerate(("q", "k", "v", "o")):
            w = wmat(f"{mn}{s}_w")
            if mn != "o":
                w = gs[:, None] * w
            wa[s - 1, :, mi] = pack_w(w).reshape(128, DC, D)
    com["wad"] = wa
    com["w1d"] = pack_w(g[3][:, None] * wmat("fc1_w")).reshape(128, DC, FH)
    com["w2d"] = pack_w(wmat("fc2_w")).reshape(128, FHC, D)

    def chunks(vec, w):
        return np.asarray(vec, f32).reshape(w, 128).T

    biasp = np.zeros((128, 66), f32)
    qb1 = b[1] @ wmat("q1_w") + np.asarray(inputs["q1_b"], f32)
    kb1 = b[1] @ wmat("k1_w") + np.asarray(inputs["k1_b"], f32)
    qb2 = b[2] @ wmat("q2_w") + np.asarray(inputs["q2_b"], f32)
    kb2 = b[2] @ wmat("k2_w") + np.asarray(inputs["k2_b"], f32)
    vb1 = b[1] @ wmat("v1_w") + np.asarray(inputs["v1_b"], f32)
    vb2 = b[2] @ wmat("v2_w") + np.asarray(inputs["v2_b"], f32)
    c1 = vb1 @ wmat("o1_w") + np.asarray(inputs["o1_b"], f32)
    c2 = vb2 @ wmat("o2_w") + np.asarray(inputs["o2_b"], f32)
    f1b = b[3] @ wmat("fc1_w") + np.asarray(inputs["fc1_b"], f32)
    biasp[:, 0:6] = chunks(qb1, DC)
    biasp[:, 6:12] = chunks(kb1, DC)
    biasp[:, 12:18] = chunks(qb2, DC)
    biasp[:, 18:24] = chunks(kb2, DC)
    biasp[:, 24:30] = chunks(c1, DC)
    biasp[:, 30:36] = chunks(c2, DC)
    biasp[:, 36:42] = chunks(np.asarray(inputs["fc2_b"], f32), DC)
    biasp[:, 42:66] = chunks(f1b, FHC)
    com["biasd"] = biasp

    cvecd = np.zeros((128, 8), bfl)
    cvecd[:, 0] = 1
    cvecd[:, 1] = 1
    cvecd[:, 4] = 1
    com["cvecd"] = cvecd
    cvec8d = np.zeros((128, 2, 16), ml_dtypes.float8_e4m3fn)
    cvec8d[:, :, 0] = 1    # hh0: m0 <- ones
    cvec8d[:, :, 3] = 1    # hh1: m1 <- ones
    com["cvec8d"] = cvec8d

    rowd = np.zeros((2, 768), bfl)
    rowd[0, 0:128] = 1                      # bcast1 row0
    rowd[0, 128:192] = 1                    # sel2 row0 -> partitions 0:64
    rowd[1, 192:256] = 1                    # sel2 row1 -> partitions 64:128
    com["rowd"] = rowd

    if with_mask:
        madd = (mask.reshape(T, T).astype(f32) - 1.0) * 1e9
        com["maskd"] = np.ascontiguousarray(
            madd.T.reshape(2, 128, T).transpose(1, 0, 2))

    in_maps = []
    for c in range(NCORE):
        shard = x[NF * c:NF * (c + 1)]                  # [16, 256, 768]
        xT = shard.reshape(NT, D).T                     # [768, 4096]
        m = dict(com)
        m["xt"] = np.ascontiguousarray(
            xT.reshape(DC, 128, 8, 512).transpose(2, 1, 0, 3)).astype(bfl)
        in_maps.append(m)
    return in_maps, with_mask


def _host_post(results, dtype):
    """results: list of 8 dicts with 'ot' [8, 128, DC, 512] f32, window order."""
    out = np.empty((NCORE * NF, T, D), dtype)
    for c, r in enumerate(results):
        o = np.asarray(r["ot"], np.float32)             # [8, 128, 6, 512]
        o = o.transpose(2, 1, 0, 3).reshape(D, NT)      # [768, 4096] wtoken order
        # wtoken = (hi, wi, f, hh, ww)
        o = o.reshape(D, 4, 4, NF, 4, 4)
        o = o.transpose(3, 1, 4, 2, 5, 0)               # (f, hi, hh, wi, ww, d)
        out[NF * c:NF * (c + 1)] = o.reshape(NF, T, D)
    return out


def kernel(**inputs) -> np.ndarray:
    from concourse.bass_utils import run_bass_kernel_spmd

    in_maps, with_mask = _host_prep(inputs)
    key = ("k", with_mask)
    if key not in _CACHE:
        _CACHE[key] = _build(with_mask)
    nc = _CACHE[key]
    res = run_bass_kernel_spmd(nc, in_maps, core_ids=list(range(NCORE)))
    return _host_post(res.results, np.asarray(inputs["x"]).dtype)


# revision 34
# speedup vs baseline: 4.3506x; 1.1960x over previous
"""Trainium2 Bass kernel for the sparse-attention block (full attn + window attn + MLP).

v2 design (per core, data-parallel over sq_len: 16 frames/core):
  - Residual stream SBUF-RESIDENT: x2_sb [128, DC, 4096] bf16, feature-major.
    Stage 1 updates it in place (frame order); stage 2 gathers window-permuted
    views on-chip (no scatter DMAs, no DRAM roundtrip between attentions).
  - All matmul operands bf16 (f32 PSUM accumulate); LN gamma/beta folded into
    the weights on the host, so on-device LN is only (x - m) * r via two
    broadcast matmuls (R, M) + fused vector ops.
  - Attention: scores [s, t] via quadrant-packed bf16 matmuls; exp on ACT;
    denominators via ones-matmuls; att@v packs head pairs into one PSUM bank
    through output-partition offsets (no zero-padded v).
  - Software-pipelined emission: LN of group k+1 is emitted before the
    attention/MLP core of group k so TensorE never starves.
"""

import math
import numpy as np

D = 768
NH = 12
HD = 64
T = 256          # tokens per attention group
NF = 16          # frames (or windows) per core
NT = NF * T      # tokens per core
NCORE = 8
DC = 6           # feature chunks (768/128)
FH = 3072        # MLP hidden
FHC = 24         # hidden chunks
EPS = 1e-5

_CACHE = {}


def _build(with_mask: bool, iters: int = 1):
    import contextlib
    from concourse import bacc
    import concourse.mybir as mybir
    import concourse.tile as tile

    f32 = mybir.dt.float32
    bf16 = mybir.dt.bfloat16
    fp8 = mybir.dt.float8e4
    DR = mybir.MatmulPerfMode.DoubleRow
    AF = mybir.ActivationFunctionType
    OP = mybir.AluOpType

    nc = bacc.Bacc("TRN2", target_bir_lowering=False, debug=False, num_devices=NCORE)

    xt = nc.dram_tensor("xt", [8, 128, DC, 512], bf16, kind="ExternalInput")
    # per stage: q, k, v, o with LN gamma folded in (rows scaled by g)
    wad = nc.dram_tensor("wad", [2, 128, 4, DC, D], bf16, kind="ExternalInput")
    w1d = nc.dram_tensor("w1d", [128, DC, FH], bf16, kind="ExternalInput")
    w2d = nc.dram_tensor("w2d", [128, FHC, D], bf16, kind="ExternalInput")
    # biasd cols: 0:6 qb1', 6:12 kb1', 12:18 qb2', 18:24 kb2', 24:30 c1',
    #             30:36 c2', 36:42 f2b, 42:66 f1b'
    biasd = nc.dram_tensor("biasd", [128, 66], f32, kind="ExternalInput")
    # cvecd cols: 0 ones(stats); 1:3 = [1,0]; 3:5 = [0,1]
    cvecd = nc.dram_tensor("cvecd", [128, 8], bf16, kind="ExternalInput")
    # cvec8d [128, 2, 16]: [:, i, 0:2] = [1,0] (hh0 DR sel); [:, i, 2:4] = [0,1]
    cvec8d = nc.dram_tensor("cvec8d", [128, 2, 16], fp8, kind="ExternalInput")
    # rowd: [:,0:128] bcast1 (row0 ones); [:,128:256] sel2; [:,256:768] zeros
    rowd = nc.dram_tensor("rowd", [2, 768], bf16, kind="ExternalInput")
    if with_mask:
        maskd = nc.dram_tensor("maskd", [128, 2, T], f32, kind="ExternalInput")

    m2 = nc.dram_tensor("m2", [8, 128, DC, 512], bf16)          # window order
    ot = nc.dram_tensor("ot", [8, 128, DC, 512], f32, kind="ExternalOutput")

    with tile.TileContext(nc) as tc:
        est = contextlib.ExitStack()
        with est:
            est.enter_context(nc.allow_low_precision(
                reason="bf16 matmuls with f32 PSUM; rel-err budget 2e-2 measured"))
            cpool = est.enter_context(tc.tile_pool(name="consts", bufs=1))
            spool = est.enter_context(tc.tile_pool(name="small", bufs=2))
            hpool = est.enter_context(tc.tile_pool(name="heads", bufs=4))

            # ---------------- constants ----------------
            cvec = cpool.tile([128, 8], bf16, tag="cvec")
            nc.sync.dma_start(cvec[:], cvecd[:])
            biasp = cpool.tile([128, 66], f32, tag="biasp")
            nc.sync.dma_start(biasp[:], biasd[:])
            cvec8 = cpool.tile([128, 2, 16], fp8, tag="cvec8")
            nc.sync.dma_start(cvec8[:], cvec8d[:])
            rowc = cpool.tile([2, 768], bf16, tag="rowc")
            nc.sync.dma_start(rowc[:], rowd[:])
            if with_mask:
                mask_sb = cpool.tile([128, 2, T], f32, tag="mask_sb")
                nc.sync.dma_start(mask_sb[:], maskd[:])

            ones_c = cvec[:, 0:1]
            bcast1 = rowc[:, 0:128]
            sel2 = rowc[:, 128:256]

            # persistent row-vector scratch (rows 1 stay zero forever)
            statrs = [cpool.tile([2, 512], bf16, tag=f"statr{i}", name=f"statr{i}")
                      for i in range(2)]
            mrzs = [cpool.tile([2, 512], bf16, tag=f"mrz{i}", name=f"mrz{i}")
                    for i in range(2)]
            for t_ in statrs + mrzs:
                nc.scalar.memzero(t_[:])

            def bias_col(idx, n=1):
                return biasp[:, idx:idx + n]

            def body(iv=None):
                st2 = contextlib.ExitStack()
                with st2:
                    x2pool = st2.enter_context(tc.tile_pool(name="resid", bufs=1))
                    wpool = st2.enter_context(tc.tile_pool(name="wts", bufs=6))
                    apool = st2.enter_context(tc.tile_pool(name="acts", bufs=2))
                    opool = st2.enter_context(tc.tile_pool(name="outs", bufs=2))
                    pp = st2.enter_context(tc.tile_pool(name="psA", bufs=8, space="PSUM"))

                    def psum():
                        return pp.tile([128, 512], f32, tag="ps", name="ps", bufs=4)

                    zero_vzp = [2]
                    x2 = x2pool.tile([128, DC, 4096], bf16, tag="x2", name="x2")
                    for p in range(8):
                        nc.sync.dma_start(x2[:, :, 512 * p:512 * (p + 1)], xt.ap()[p])

                    wt = {}
                    for s in range(2):
                        for mi, mn in enumerate("qkvo"):
                            w_ = wpool.tile([128, DC, D], bf16, tag="w", name=f"w{mn}{s}")
                            nc.gpsimd.dma_start(w_[:], wad.ap()[s, :, mi])
                            wt[(s, mn)] = w_

                    # ---------------- layernorm over 512 tokens (split emission) ----------------
                    def ln_stats(xviews, pidx, psum_fn, sq_act=False):
                        """Stats matmuls + row chain -> statr/mrz rows. Releases PSUM fast."""
                        statr, mrz = statrs[pidx % 2], mrzs[pidx % 2]
                        st1, st2_ = psum_fn(), psum_fn()
                        for c in range(DC):
                            x2t = spool.tile([128, 512], bf16, tag="x2t", name="x2t", bufs=3)
                            if sq_act:
                                nc.scalar.activation(x2t[:], xviews[c], AF.Square,
                                                     bias=0.0, scale=1.0)
                            else:
                                nc.vector.tensor_tensor(x2t[:], xviews[c], xviews[c], OP.mult)
                            nc.tensor.matmul(st1[0:1, :], ones_c, xviews[c],
                                             start=(c == 0), stop=(c == DC - 1))
                            nc.tensor.matmul(st2_[0:1, :], ones_c, x2t[:],
                                             start=(c == 0), stop=(c == DC - 1))
                        t_m = spool.tile([2, 512], f32, tag="vec", name="t_m", bufs=4)
                        t_e = spool.tile([2, 512], f32, tag="vec", name="t_e", bufs=4)
                        # m_neg = -Sx/D ; E2 = Sx2/D ; var = E2 - m^2
                        # r = 1/sqrt(var+eps) = exp(-0.5*log(var+eps)) -- stays in the
                        # exp/log ACT table set (no per-group table reload)
                        nc.vector.tensor_scalar_mul(t_m[0:1, :], st1[0:1, :], -1.0 / D)
                        nc.vector.tensor_scalar_mul(t_e[0:1, :], st2_[0:1, :], 1.0 / D)
                        nc.vector.tensor_tensor(mrz[0:1, :], t_m[0:1, :], t_m[0:1, :], OP.mult)
                        nc.vector.tensor_tensor(t_e[0:1, :], t_e[0:1, :], mrz[0:1, :], OP.subtract)
                        nc.vector.tensor_scalar_add(t_e[0:1, :], t_e[0:1, :], EPS)
                        nc.scalar.activation(t_e[0:1, :], t_e[0:1, :], AF.Ln,
                                             bias=0.0, scale=1.0)
                        nc.scalar.activation(statr[0:1, :], t_e[0:1, :], AF.Exp,
                                             bias=0.0, scale=-0.5)
                        nc.vector.tensor_tensor(mrz[0:1, :], t_m[0:1, :],
                                                statr[0:1, :], OP.mult)

                    def ln_finish(xall, a_out, pidx, psum_fn):
                        """Broadcast R/M + normalize into a_out.
                        xall: [128, DC, 512] AP; a_out: [128, DC, 512] tile."""
                        statr, mrz = statrs[pidx % 2], mrzs[pidx % 2]
                        rps, mps = psum_fn(), psum_fn()
                        nc.tensor.matmul(rps[:, :], bcast1, statr[:, :], start=True, stop=True)
                        nc.tensor.matmul(mps[:, :], bcast1, mrz[:, :], start=True, stop=True)
                        rsb = spool.tile([128, 512], bf16, tag="rmsb", name="rsb", bufs=4)
                        msb = spool.tile([128, 512], bf16, tag="rmsb", name="msb", bufs=4)
                        nc.scalar.copy(rsb[:], rps[:, :])
                        nc.scalar.copy(msb[:], mps[:, :])
                        for c in range(DC):
                            tmp = spool.tile([128, 512], bf16, tag="x2t", name="lntmp", bufs=3)
                            nc.vector.tensor_tensor(tmp[:], xall[:, c, :], rsb[:], OP.mult)
                            nc.vector.tensor_tensor(a_out[:, c, :], tmp[:], msb[:], OP.add)

                    # ---------------- attention core for one 512-token pair ----------------
                    def attn_qkv(stage, a1):
                        s = stage - 1
                        wq, wk, wv, wo = (wt[(s, m)] for m in "qkvo")
                        qb = bias_col(0 if stage == 1 else 12, DC)
                        kb = bias_col(6 if stage == 1 else 18, DC)

                        qT = apool.tile([128, DC, 512], bf16, tag="qT", name="qT", bufs=1)
                        kT = apool.tile([128, DC, 512], bf16, tag="kT", name="kT", bufs=1)
                        for oc in range(DC):
                            ps = psum()
                            for c in range(DC):
                                nc.tensor.matmul(ps[:, :], wq[:, c, 128 * oc:128 * (oc + 1)],
                                                 a1[:, c, :], start=(c == 0), stop=(c == DC - 1))
                            if stage == 1:
                                nc.scalar.activation(qT[:, oc, :], ps[:, :], AF.Identity,
                                                     bias=qb[:, oc:oc + 1], scale=1.0)
                            else:
                                nc.vector.tensor_scalar_add(qT[:, oc, :], ps[:, :],
                                                            qb[:, oc:oc + 1])
                            ps = psum()
                            for c in range(DC):
                                nc.tensor.matmul(ps[:, :], wk[:, c, 128 * oc:128 * (oc + 1)],
                                                 a1[:, c, :], start=(c == 0), stop=(c == DC - 1))
                            nc.vector.tensor_scalar_add(kT[:, oc, :], ps[:, :], kb[:, oc:oc + 1])
                        # v in [s, d] orientation, zero-padded per head pair:
                        # vzp[:, f, hp, hh, sc2, :]: hh=0 -> [v|0], hh=1 -> [0|v]
                        vzp = apool.tile([128, 2, 6, 2, 2, 128], fp8, tag="vv",
                                         name="vzp", bufs=2)
                        if zero_vzp[0] > 0:
                            zero_vzp[0] -= 1
                            nc.scalar.memzero(
                                vzp[:].rearrange("p a b c d e -> p (a b c d e)"))
                        for sc in range(4):
                            f, sc2 = sc // 2, sc % 2
                            for half in range(2):
                                ps = psum()
                                for c in range(DC):
                                    nc.tensor.matmul(ps[:, 0:384],
                                                     a1[:, c, 128 * sc:128 * (sc + 1)],
                                                     wv[:, c, 384 * half:384 * (half + 1)],
                                                     start=(c == 0), stop=(c == DC - 1))
                                psv = ps[:, 0:384].rearrange(
                                    "p (hp hh d) -> p hp hh d", hp=3, hh=2)
                                for hh in range(2):
                                    nc.vector.tensor_copy(
                                        vzp[:, f, 3 * half:3 * (half + 1), hh, sc2,
                                            64 * hh:64 * hh + 64],
                                        psv[:, :, hh, :])
                        return qT, kT, vzp

                    def attn_rest(stage, qkv, xviews, out_writer):
                        s = stage - 1
                        qT, kT, vzp = qkv
                        wo = wt[(s, "o")]
                        cb = bias_col(24 if stage == 1 else 30, DC)
                        yT = apool.tile([128, DC, 512], bf16, tag="yT", name="yT", bufs=2)
                        for f in range(2):
                            toff = 256 * f
                            for hp in range(6):
                                sps = pp.tile([128, 2, 512], f32, tag="ps2", name="sps", bufs=2)
                                for sc2 in range(2):
                                    for hh in range(2):
                                        nc.tensor.matmul(
                                            sps[:, hh, 256 * sc2:256 * (sc2 + 1)],
                                            kT[64 * hh:64 * (hh + 1), hp,
                                               toff + 128 * sc2:toff + 128 * (sc2 + 1)],
                                            qT[64 * hh:64 * (hh + 1), hp, toff:toff + 256],
                                            start=(sc2 == 0), stop=(sc2 == 1),
                                            tile_position=(64 * hh, 0))
                                if with_mask and stage == 1:
                                    for hh in range(2):
                                        nc.vector.tensor_tensor(
                                            sps[:, hh, :], sps[:, hh, :],
                                            mask_sb[:].rearrange("p s t -> p (s t)"), OP.add)
                                attb = hpool.tile([128, 2, 2, T], fp8, tag="att", name="attb")
                                nc.scalar.activation(
                                    attb[:].rearrange("p h s t -> p (h s t)"),
                                    sps[:].rearrange("p h x -> p (h x)"),
                                    AF.Exp, bias=0.0, scale=1.0 / math.sqrt(HD))
                                att = [attb[:, 0], attb[:, 1]]
                                pd = psum()
                                for hh in range(2):
                                    nc.tensor.matmul(
                                        pd[0:2, 0:T],
                                        cvec8[:, :, 2 * hh:2 * hh + 2],
                                        att[hh], perf_mode=DR,
                                        start=(hh == 0), stop=(hh == 1))
                                yb = psum()
                                for hh in range(2):
                                    nc.tensor.matmul(
                                        yb[:, 0:T],
                                        vzp[:, f, hp, hh],
                                        att[hh], perf_mode=DR,
                                        start=(hh == 0), stop=(hh == 1))
                                pdr = spool.tile([2, 512], bf16, tag="pdr", name="pdr", bufs=4)
                                nc.vector.reciprocal(pdr[0:2, 0:T], pd[0:2, 0:T])
                                nc.tensor.matmul(pd[:, T:2 * T], sel2, pdr[0:2, 0:T],
                                                 start=True, stop=True)
                                nc.vector.tensor_tensor(yT[:, hp, toff:toff + 256],
                                                        yb[:, 0:T], pd[:, T:2 * T], OP.mult)

                        for ec in range(DC):
                            ps = psum()
                            for hc in range(DC):
                                nc.tensor.matmul(ps[:, :], wo[:, hc, 128 * ec:128 * (ec + 1)],
                                                 yT[:, hc, :], start=(hc == 0), stop=(hc == DC - 1))
                            out_writer(ec, ps, cb, xviews)

                    # ================= stage 1 (frame order, in-place on x2) =================
                    def x2views(p):
                        return [x2[:, c, 512 * p:512 * (p + 1)] for c in range(DC)]

                    def s1_writer_for(p):
                        def wr(ec, ps, cb, xviews):
                            nc.vector.scalar_tensor_tensor(
                                x2[:, ec, 512 * p:512 * (p + 1)], ps[:, :],
                                cb[:, ec:ec + 1], xviews[ec], OP.add, OP.add)
                        return wr

                    a1s = {}
                    for p in range(9):
                        if p < 8:
                            ln_stats(x2views(p), p, psum)
                        if p >= 1:
                            qkv = attn_qkv(1, a1s.pop(p - 1))
                        if p < 8:
                            a1 = apool.tile([128, DC, 512], bf16, tag="a1", name="a1", bufs=2)
                            ln_finish(x2[:, :, 512 * p:512 * (p + 1)], a1, p, psum)
                            a1s[p] = a1
                        if p >= 1:
                            attn_rest(1, qkv, x2views(p - 1), s1_writer_for(p - 1))

                    # ================= stage 2 (window order, gather from x2) =================
                    # x2 token idx = f*256 + (4hi+hh)*16 + 4wi+ww ; window w=(hi,wi)
                    x2w = x2[:].rearrange("p c (f hi hh wi ww) -> p c hi wi f hh ww",
                                          f=16, hi=4, hh=4, wi=4)

                    def ln_stats_s2(wp, pidx):
                        """Stage-2 stats direct from x2 (strided views, pre-gather)."""
                        statr, mrz = statrs[pidx % 2], mrzs[pidx % 2]
                        st1, st2_ = psum(), psum()
                        for c in range(DC):
                            for j in range(2):
                                w = 2 * wp + j
                                src_ = x2w[:, c, w // 4, w % 4]   # [128, 16, 4, 4]
                                x2t = spool.tile([128, 256], bf16, tag="x2t2",
                                                 name="x2t2", bufs=3)
                                x2tv = x2t[:].rearrange("p (f hh ww) -> p f hh ww",
                                                        f=16, hh=4)
                                nc.vector.tensor_tensor(x2tv, src_, src_, OP.mult)
                                nc.tensor.matmul(st1[0:1, 256 * j:256 * (j + 1)],
                                                 ones_c, src_,
                                                 start=(c == 0), stop=(c == DC - 1))
                                nc.tensor.matmul(st2_[0:1, 256 * j:256 * (j + 1)],
                                                 ones_c, x2t[:],
                                                 start=(c == 0), stop=(c == DC - 1))
                        t_m = spool.tile([2, 512], f32, tag="vec", name="t_m2", bufs=4)
                        t_e = spool.tile([2, 512], f32, tag="vec", name="t_e2", bufs=4)
                        nc.vector.tensor_scalar_mul(t_m[0:1, :], st1[0:1, :], -1.0 / D)
                        nc.vector.tensor_scalar_mul(t_e[0:1, :], st2_[0:1, :], 1.0 / D)
                        nc.vector.tensor_tensor(mrz[0:1, :], t_m[0:1, :], t_m[0:1, :], OP.mult)
                        nc.vector.tensor_tensor(t_e[0:1, :], t_e[0:1, :], mrz[0:1, :], OP.subtract)
                        nc.vector.tensor_scalar_add(t_e[0:1, :], t_e[0:1, :], EPS)
                        nc.scalar.activation(t_e[0:1, :], t_e[0:1, :], AF.Ln,
                                             bias=0.0, scale=1.0)
                        nc.scalar.activation(statr[0:1, :], t_e[0:1, :], AF.Exp,
                                             bias=0.0, scale=-0.5)
                        nc.vector.tensor_tensor(mrz[0:1, :], t_m[0:1, :],
                                                statr[0:1, :], OP.mult)

                    def gather(wp):
                        xs2 = opool.tile([128, DC, 512], bf16, tag="xs", name="xs2", bufs=2)
                        for c in range(DC):
                            for j in range(2):
                                w = 2 * wp + j
                                src = x2w[:, c, w // 4, w % 4]  # [128, 16, 4, 4]
                                dst = xs2[:, c, 256 * j:256 * (j + 1)]
                                nc.scalar.copy(
                                    dst.rearrange("p (f hh ww) -> p f hh ww", f=16, hh=4),
                                    src)
                        return xs2

                    def s2_writer_for(oo2):
                        def wr(ec, ps, cb, xviews):
                            nc.vector.scalar_tensor_tensor(
                                oo2[:, ec, :], ps[:, :], cb[:, ec:ec + 1],
                                xviews[ec], OP.add, OP.add)
                        return wr

                    prev = None
                    for wp in range(9):
                        if wp < 8:
                            ln_stats_s2(wp, wp)
                            xs2 = gather(wp)
                        if wp >= 1:
                            qkv = attn_qkv(2, prev[1])
                        if wp < 8:
                            a1 = apool.tile([128, DC, 512], bf16, tag="a1", name="a1b", bufs=2)
                            ln_finish(xs2[:], a1, wp, psum)
                        if wp >= 1:
                            p_ = wp - 1
                            oo2 = opool.tile([128, DC, 512], bf16, tag="oo2", name="oo2", bufs=2)
                            attn_rest(2, qkv, [prev[0][:, c, :] for c in range(DC)],
                                      s2_writer_for(oo2))
                            nc.sync.dma_start(m2.ap()[p_], oo2[:])
                        if wp < 8:
                            prev = (xs2, a1)

                # ================= MLP (window order slabs from m2) =================
                st3 = contextlib.ExitStack()
                with st3:
                    wmpool = st3.enter_context(tc.tile_pool(name="wtsm", bufs=2))
                    mpool = st3.enter_context(tc.tile_pool(name="mact", bufs=2))
                    gpool = st3.enter_context(tc.tile_pool(name="gm", bufs=1))
                    ppB = st3.enter_context(tc.tile_pool(name="psB", bufs=8, space="PSUM"))

                    def psumB():
                        return ppB.tile([128, 512], f32, tag="psb", name="psb")

                    w1 = wmpool.tile([128, DC, FH], bf16, tag="wm", name="w1")
                    nc.gpsimd.dma_start(w1[:], w1d.ap())
                    w2 = wmpool.tile([128, FHC, D], bf16, tag="wm", name="w2")
                    nc.gpsimd.dma_start(w2[:], w2d.ap())
                    f1b = bias_col(42, FHC)
                    f2b = bias_col(36, DC)

                    def ln512B(xviews, a_out, pidx):
                        statr, mrz = statrs[pidx % 2], mrzs[pidx % 2]
                        st1, st2_ = psumB(), psumB()
                        for c in range(DC):
                            x2t = spool.tile([128, 512], bf16, tag="x2t", name="x2tB", bufs=3)
                            nc.scalar.activation(x2t[:], xviews[c], AF.Square,
                                                 bias=0.0, scale=1.0)
                            nc.tensor.matmul(st1[0:1, :], ones_c, xviews[c],
                                             start=(c == 0), stop=(c == DC - 1))
                            nc.tensor.matmul(st2_[0:1, :], ones_c, x2t[:],
                                             start=(c == 0), stop=(c == DC - 1))
                        t_m = spool.tile([2, 512], f32, tag="vec", name="t_mB", bufs=4)
                        t_e = spool.tile([2, 512], f32, tag="vec", name="t_eB", bufs=4)
                        nc.vector.tensor_scalar_mul(t_m[0:1, :], st1[0:1, :], -1.0 / D)
                        nc.vector.tensor_scalar_mul(t_e[0:1, :], st2_[0:1, :], 1.0 / D)
                        nc.vector.tensor_tensor(mrz[0:1, :], t_m[0:1, :], t_m[0:1, :], OP.mult)
                        nc.vector.tensor_tensor(t_e[0:1, :], t_e[0:1, :], mrz[0:1, :], OP.subtract)
                        nc.vector.tensor_scalar_add(t_e[0:1, :], t_e[0:1, :], EPS)
                        nc.scalar.activation(t_e[0:1, :], t_e[0:1, :], AF.Sqrt,
                                             bias=0.0, scale=1.0)
                        nc.vector.reciprocal(statr[0:1, :], t_e[0:1, :])
                        nc.vector.tensor_tensor(mrz[0:1, :], t_m[0:1, :],
                                                statr[0:1, :], OP.mult)
                        rps, mps = psumB(), psumB()
                        nc.tensor.matmul(rps[:, :], bcast1, statr[:, :], start=True, stop=True)
                        nc.tensor.matmul(mps[:, :], bcast1, mrz[:, :], start=True, stop=True)
                        rsb = spool.tile([128, 512], bf16, tag="rmsb", name="rsbB", bufs=4)
                        msb = spool.tile([128, 512], bf16, tag="rmsb", name="msbB", bufs=4)
                        nc.scalar.copy(rsb[:], rps[:, :])
                        nc.scalar.copy(msb[:], mps[:, :])
                        for c in range(DC):
                            tmp = spool.tile([128, 512], bf16, tag="x2t", name="lntmpB", bufs=3)
                            nc.vector.tensor_tensor(tmp[:], xviews[c], rsb[:], OP.mult)
                            nc.vector.tensor_tensor(a_out[:, c, :], tmp[:], msb[:], OP.add)

                    def mlp_fc1(h3):
                        g1 = gpool.tile([128, FHC, 512], bf16, tag="g1", name="g1", bufs=2)
                        for oc in range(FHC):
                            ps1 = psumB()
                            for c in range(DC):
                                nc.tensor.matmul(ps1[:, :], w1[:, c, 128 * oc:128 * (oc + 1)],
                                                 h3[:, c, :], start=(c == 0), stop=(c == DC - 1))
                            nc.scalar.activation(g1[:, oc, :], ps1[:, :], AF.Gelu,
                                                 bias=f1b[:, oc:oc + 1], scale=1.0)
                        return g1

                    def mlp_fc2(xs3, g1, sl):
                        oo = mpool.tile([128, DC, 512], f32, tag="oo", name="oo", bufs=2)
                        for ec in range(DC):
                            zps = psumB()
                            for oc in range(FHC):
                                nc.tensor.matmul(zps[:, :], w2[:, oc, 128 * ec:128 * (ec + 1)],
                                                 g1[:, oc, :], start=(oc == 0), stop=(oc == FHC - 1))
                            nc.vector.scalar_tensor_tensor(
                                oo[:, ec, :], zps[:, :], f2b[:, ec:ec + 1],
                                xs3[:, ec, :], OP.add, OP.add)
                        nc.sync.dma_start(ot.ap()[sl], oo[:])

                    prevm = None
                    for sl in range(9):
                        if sl >= 1:
                            g1p = mlp_fc1(prevm[1])
                        if sl < 8:
                            xs3 = mpool.tile([128, DC, 512], bf16, tag="xs3", name="xs3", bufs=2)
                            nc.sync.dma_start(xs3[:], m2.ap()[sl])
                            ln_stats([xs3[:, c, :] for c in range(DC)], sl, psumB)
                        if sl >= 1:
                            mlp_fc2(prevm[0], g1p, sl - 1)
                        if sl < 8:
                            h3 = mpool.tile([128, DC, 512], bf16, tag="h3", name="h3", bufs=2)
                            ln_finish(xs3[:], h3, sl, psumB)
                            prevm = (xs3, h3)

            if iters == 1:
                body()
            else:
                with tc.For_i(0, iters, 1) as iv:
                    body(iv)

    nc.compile()
    return nc


def _host_prep(inputs):
    """Build per-core input maps (bf16 weights with LN folding)."""
    import ml_dtypes
    f32 = np.float32
    bfl = ml_dtypes.bfloat16
    x = np.asarray(inputs["x"], f32)
    mask = np.asarray(inputs["mask"])
    with_mask = not bool((mask == 1).all())

    g = {i: np.asarray(inputs[f"ln{i}_g"], f32) for i in (1, 2, 3)}
    b = {i: np.asarray(inputs[f"ln{i}_b"], f32) for i in (1, 2, 3)}

    def wmat(key):
        return np.asarray(inputs[key], f32)

    def pack_w(w):
        # [in, out] -> [128, kc, out]
        kc = w.shape[0] // 128
        return np.ascontiguousarray(
            w.reshape(kc, 128, w.shape[1]).transpose(1, 0, 2)).astype(bfl)

    com = {}
    wa = np.zeros((2, 128, 4, DC, D), bfl)
    for s in (1, 2):
        gs, bs = g[s], b[s]
        for mi, mn in enumerate(("q", "k", "v", "o")):
            w = wmat(f"{mn}{s}_w")
            if mn != "o":
                w = gs[:, None] * w
            wa[s - 1, :, mi] = pack_w(w).reshape(128, DC, D)
    com["wad"] = wa
    com["w1d"] = pack_w(g[3][:, None] * wmat("fc1_w")).reshape(128, DC, FH)
    com["w2d"] = pack_w(wmat("fc2_w")).reshape(128, FHC, D)

    def chunks(vec, w):
        return np.asarray(vec, f32).reshape(w, 128).T

    biasp = np.zeros((128, 66), f32)
    qb1 = b[1] @ wmat("q1_w") + np.asarray(inputs["q1_b"], f32)
    kb1 = b[1] @ wmat("k1_w") + np.asarray(inputs["k1_b"], f32)
    qb2 = b[2] @ wmat("q2_w") + np.asarray(inputs["q2_b"], f32)
    kb2 = b[2] @ wmat("k2_w") + np.asarray(inputs["k2_b"], f32)
    vb1 = b[1] @ wmat("v1_w") + np.asarray(inputs["v1_b"], f32)
    vb2 = b[2] @ wmat("v2_w") + np.asarray(inputs["v2_b"], f32)
    c1 = vb1 @ wmat("o1_w") + np.asarray(inputs["o1_b"], f32)
    c2 = vb2 @ wmat("o2_w") + np.asarray(inputs["o2_b"], f32)
    f1b = b[3] @ wmat("fc1_w") + np.asarray(inputs["fc1_b"], f32)
    biasp[:, 0:6] = chunks(qb1, DC)
    biasp[:, 6:12] = chunks(kb1, DC)
    biasp[:, 12:18] = chunks(qb2, DC)
    biasp[:, 18:24] = chunks(kb2, DC)
    biasp[:, 24:30] = chunks(c1, DC)
    biasp[:, 30:36] = chunks(c2, DC)
    biasp[:, 36:42] = chunks(np.asarray(inputs["fc2_b"], f32), DC)
    biasp[:, 42:66] = chunks(f1b, FHC)
    com["biasd"] = biasp

    cvecd = np.zeros((128, 8), bfl)
    cvecd[:, 0] = 1
    cvecd[:, 1] = 1
    cvecd[:, 4] = 1
    com["cvecd"] = cvecd
    cvec8d = np.zeros((128, 2, 16), ml_dtypes.float8_e4m3fn)
    cvec8d[:, :, 0] = 1    # hh0: m0 <- ones
    cvec8d[:, :, 3] = 1    # hh1: m1 <- ones
    com["cvec8d"] = cvec8d

    rowd = np.zeros((2, 768), bfl)
    rowd[0, 0:128] = 1                      # bcast1 row0
    rowd[0, 128:192] = 1                    # sel2 row0 -> partitions 0:64
    rowd[1, 192:256] = 1                    # sel2 row1 -> partitions 64:128
    com["rowd"] = rowd

    if with_mask:
        madd = (mask.reshape(T, T).astype(f32) - 1.0) * 1e9
        com["maskd"] = np.ascontiguousarray(
            madd.T.reshape(2, 128, T).transpose(1, 0, 2))

    in_maps = []
    for c in range(NCORE):
        shard = x[NF * c:NF * (c + 1)]                  # [16, 256, 768]
        xT = shard.reshape(NT, D).T                     # [768, 4096]
        m = dict(com)
        m["xt"] = np.ascontiguousarray(
            xT.reshape(DC, 128, 8, 512).transpose(2, 1, 0, 3)).astype(bfl)
        in_maps.append(m)
    return in_maps, with_mask


def _host_post(results, dtype):
    """results: list of 8 dicts with 'ot' [8, 128, DC, 512] f32, window order."""
    out = np.empty((NCORE * NF, T, D), dtype)
    for c, r in enumerate(results):
        o = np.asarray(r["ot"], np.float32)             # [8, 128, 6, 512]
        o = o.transpose(2, 1, 0, 3).reshape(D, NT)      # [768, 4096] wtoken order
        # wtoken = (hi, wi, f, hh, ww)
        o = o.reshape(D, 4, 4, NF, 4, 4)
        o = o.transpose(3, 1, 4, 2, 5, 0)               # (f, hi, hh, wi, ww, d)
        out[NF * c:NF * (c + 1)] = o.reshape(NF, T, D)
    return out


def kernel(**inputs) -> np.ndarray:
    from concourse.bass_utils import run_bass_kernel_spmd

    in_maps, with_mask = _host_prep(inputs)
    key = ("k", with_mask)
    if key not in _CACHE:
        _CACHE[key] = _build(with_mask)
    nc = _CACHE[key]
    res = run_bass_kernel_spmd(nc, in_maps, core_ids=list(range(NCORE)))
    return _host_post(res.results, np.asarray(inputs["x"]).dtype)


# revision 38
# speedup vs baseline: 4.3541x; 1.0008x over previous
"""Trainium2 Bass kernel for the sparse-attention block (full attn + window attn + MLP).

v2 design (per core, data-parallel over sq_len: 16 frames/core):
  - Residual stream SBUF-RESIDENT: x2_sb [128, DC, 4096] bf16, feature-major.
    Stage 1 updates it in place (frame order); stage 2 gathers window-permuted
    views on-chip (no scatter DMAs, no DRAM roundtrip between attentions).
  - All matmul operands bf16 (f32 PSUM accumulate); LN gamma/beta folded into
    the weights on the host, so on-device LN is only (x - m) * r via two
    broadcast matmuls (R, M) + fused vector ops.
  - Attention: scores [s, t] via quadrant-packed bf16 matmuls; one merged exp
    per head pair (2-bank PSUM read); att@v and softmax denominators run as
    fp8e4m3 DoubleRow matmuls (contraction 256 at 0.5 cyc/row) over a
    zero-padded v layout -- safe because softmax structure bounds the error.
  - Software-pipelined emission: LN stats/row-chain of group k+1 are emitted
    around the projection/attention work of group k so the in-order TensorE
    never starves on the LN chain.
"""

import math
import numpy as np

D = 768
NH = 12
HD = 64
T = 256          # tokens per attention group
NF = 16          # frames (or windows) per core
NT = NF * T      # tokens per core
NCORE = 8
DC = 6           # feature chunks (768/128)
FH = 3072        # MLP hidden
FHC = 24         # hidden chunks
EPS = 1e-5

_CACHE = {}


def _build(with_mask: bool, iters: int = 1):
    import contextlib
    from concourse import bacc
    import concourse.mybir as mybir
    import concourse.tile as tile

    f32 = mybir.dt.float32
    bf16 = mybir.dt.bfloat16
    fp8 = mybir.dt.float8e4
    DR = mybir.MatmulPerfMode.DoubleRow
    AF = mybir.ActivationFunctionType
    OP = mybir.AluOpType

    nc = bacc.Bacc("TRN2", target_bir_lowering=False, debug=False, num_devices=NCORE)

    xt = nc.dram_tensor("xt", [8, 128, DC, 512], bf16, kind="ExternalInput")
    # per stage: q, k, v, o with LN gamma folded in (rows scaled by g)
    wad = nc.dram_tensor("wad", [2, 128, 4, DC, D], bf16, kind="ExternalInput")
    w1d = nc.dram_tensor("w1d", [128, DC, FH], bf16, kind="ExternalInput")
    w2d = nc.dram_tensor("w2d", [128, FHC, D], bf16, kind="ExternalInput")
    # biasd cols: 0:6 qb1', 6:12 kb1', 12:18 qb2', 18:24 kb2', 24:30 c1',
    #             30:36 c2', 36:42 f2b, 42:66 f1b'
    biasd = nc.dram_tensor("biasd", [128, 66], f32, kind="ExternalInput")
    # cvecd cols: 0 ones(stats); 1:3 = [1,0]; 3:5 = [0,1]
    cvecd = nc.dram_tensor("cvecd", [128, 8], bf16, kind="ExternalInput")
    # cvec8d [128, 2, 16]: [:, i, 0:2] = [1,0] (hh0 DR sel); [:, i, 2:4] = [0,1]
    cvec8d = nc.dram_tensor("cvec8d", [128, 2, 16], fp8, kind="ExternalInput")
    # rowd: [:,0:128] bcast1 (row0 ones); [:,128:256] sel2; [:,256:768] zeros
    rowd = nc.dram_tensor("rowd", [2, 768], bf16, kind="ExternalInput")
    if with_mask:
        maskd = nc.dram_tensor("maskd", [128, 2, T], f32, kind="ExternalInput")

    m2 = nc.dram_tensor("m2", [8, 128, DC, 512], bf16)          # window order
    ot = nc.dram_tensor("ot", [8, 128, DC, 512], f32, kind="ExternalOutput")

    with tile.TileContext(nc) as tc:
        est = contextlib.ExitStack()
        with est:
            est.enter_context(nc.allow_low_precision(
                reason="bf16 matmuls with f32 PSUM; rel-err budget 2e-2 measured"))
            cpool = est.enter_context(tc.tile_pool(name="consts", bufs=1))
            spool = est.enter_context(tc.tile_pool(name="small", bufs=2))
            hpool = est.enter_context(tc.tile_pool(name="heads", bufs=4))

            # ---------------- constants ----------------
            cvec = cpool.tile([128, 8], bf16, tag="cvec")
            nc.sync.dma_start(cvec[:], cvecd[:])
            biasp = cpool.tile([128, 66], f32, tag="biasp")
            nc.sync.dma_start(biasp[:], biasd[:])
            cvec8 = cpool.tile([128, 2, 16], fp8, tag="cvec8")
            nc.sync.dma_start(cvec8[:], cvec8d[:])
            rowc = cpool.tile([2, 768], bf16, tag="rowc")
            nc.sync.dma_start(rowc[:], rowd[:])
            if with_mask:
                mask_sb = cpool.tile([128, 2, T], f32, tag="mask_sb")
                nc.sync.dma_start(mask_sb[:], maskd[:])

            ones_c = cvec[:, 0:1]
            bcast1 = rowc[:, 0:128]
            sel2 = rowc[:, 128:256]

            # persistent row-vector scratch (rows 1 stay zero forever)
            statrs = [cpool.tile([2, 512], bf16, tag=f"statr{i}", name=f"statr{i}")
                      for i in range(2)]
            mrzs = [cpool.tile([2, 512], bf16, tag=f"mrz{i}", name=f"mrz{i}")
                    for i in range(2)]
            for t_ in statrs + mrzs:
                nc.scalar.memzero(t_[:])

            def bias_col(idx, n=1):
                return biasp[:, idx:idx + n]

            def body(iv=None):
                st2 = contextlib.ExitStack()
                with st2:
                    x2pool = st2.enter_context(tc.tile_pool(name="resid", bufs=1))
                    wpool = st2.enter_context(tc.tile_pool(name="wts", bufs=6))
                    apool = st2.enter_context(tc.tile_pool(name="acts", bufs=2))
                    opool = st2.enter_context(tc.tile_pool(name="outs", bufs=2))
                    pp = st2.enter_context(tc.tile_pool(name="psA", bufs=8, space="PSUM"))

                    def psum():
                        return pp.tile([128, 512], f32, tag="ps", name="ps", bufs=4)

                    zero_vzp = [2]
                    x2 = x2pool.tile([128, DC, 4096], bf16, tag="x2", name="x2")
                    for p in range(8):
                        nc.sync.dma_start(x2[:, :, 512 * p:512 * (p + 1)], xt.ap()[p])

                    wt = {}
                    for s in range(2):
                        for mi, mn in enumerate("qkvo"):
                            w_ = wpool.tile([128, DC, D], bf16, tag="w", name=f"w{mn}{s}")
                            nc.gpsimd.dma_start(w_[:], wad.ap()[s, :, mi])
                            wt[(s, mn)] = w_

                    # ---------------- layernorm over 512 tokens (split emission) ----------------
                    def ln_stats(xviews, pidx, psum_fn, sq_act=False):
                        """Stats matmuls + row chain -> statr/mrz rows. Releases PSUM fast."""
                        statr, mrz = statrs[pidx % 2], mrzs[pidx % 2]
                        st1, st2_ = psum_fn(), psum_fn()
                        for c in range(DC):
                            x2t = spool.tile([128, 512], bf16, tag="x2t", name="x2t", bufs=3)
                            if sq_act:
                                nc.scalar.activation(x2t[:], xviews[c], AF.Square,
                                                     bias=0.0, scale=1.0)
                            else:
                                nc.vector.tensor_tensor(x2t[:], xviews[c], xviews[c], OP.mult)
                            nc.tensor.matmul(st1[0:1, :], ones_c, xviews[c],
                                             start=(c == 0), stop=(c == DC - 1))
                            nc.tensor.matmul(st2_[0:1, :], ones_c, x2t[:],
                                             start=(c == 0), stop=(c == DC - 1))
                        t_m = spool.tile([2, 512], f32, tag="vec", name="t_m", bufs=4)
                        t_e = spool.tile([2, 512], f32, tag="vec", name="t_e", bufs=4)
                        # m_neg = -Sx/D ; E2 = Sx2/D ; var = E2 - m^2
                        # r = 1/sqrt(var+eps) = exp(-0.5*log(var+eps)) -- stays in the
                        # exp/log ACT table set (no per-group table reload)
                        nc.vector.tensor_scalar_mul(t_m[0:1, :], st1[0:1, :], -1.0 / D)
                        nc.vector.tensor_scalar_mul(t_e[0:1, :], st2_[0:1, :], 1.0 / D)
                        nc.vector.tensor_tensor(mrz[0:1, :], t_m[0:1, :], t_m[0:1, :], OP.mult)
                        nc.vector.tensor_tensor(t_e[0:1, :], t_e[0:1, :], mrz[0:1, :], OP.subtract)
                        nc.vector.tensor_scalar_add(t_e[0:1, :], t_e[0:1, :], EPS)
                        nc.scalar.activation(t_e[0:1, :], t_e[0:1, :], AF.Ln,
                                             bias=0.0, scale=1.0)
                        nc.scalar.activation(statr[0:1, :], t_e[0:1, :], AF.Exp,
                                             bias=0.0, scale=-0.5)
                        nc.vector.tensor_tensor(mrz[0:1, :], t_m[0:1, :],
                                                statr[0:1, :], OP.mult)

                    def ln_finish(xall, a_out, pidx, psum_fn):
                        """Broadcast R/M + normalize into a_out.
                        xall: [128, DC, 512] AP; a_out: [128, DC, 512] tile."""
                        statr, mrz = statrs[pidx % 2], mrzs[pidx % 2]
                        rps, mps = psum_fn(), psum_fn()
                        nc.tensor.matmul(rps[:, :], bcast1, statr[:, :], start=True, stop=True)
                        nc.tensor.matmul(mps[:, :], bcast1, mrz[:, :], start=True, stop=True)
                        rsb = spool.tile([128, 512], bf16, tag="rmsb", name="rsb", bufs=4)
                        msb = spool.tile([128, 512], bf16, tag="rmsb", name="msb", bufs=4)
                        nc.scalar.copy(rsb[:], rps[:, :])
                        nc.scalar.copy(msb[:], mps[:, :])
                        for c in range(DC):
                            tmp = spool.tile([128, 512], bf16, tag="x2t", name="lntmp", bufs=3)
                            nc.vector.tensor_tensor(tmp[:], xall[:, c, :], rsb[:], OP.mult)
                            nc.vector.tensor_tensor(a_out[:, c, :], tmp[:], msb[:], OP.add)

                    # ---------------- attention core for one 512-token pair ----------------
                    def attn_qkv(stage, a1):
                        s = stage - 1
                        wq, wk, wv, wo = (wt[(s, m)] for m in "qkvo")
                        qb = bias_col(0 if stage == 1 else 12, DC)
                        kb = bias_col(6 if stage == 1 else 18, DC)

                        qT = apool.tile([128, DC, 512], bf16, tag="qT", name="qT", bufs=1)
                        kT = apool.tile([128, DC, 512], bf16, tag="kT", name="kT", bufs=1)
                        for oc in range(DC):
                            ps = psum()
                            for c in range(DC):
                                nc.tensor.matmul(ps[:, :], wq[:, c, 128 * oc:128 * (oc + 1)],
                                                 a1[:, c, :], start=(c == 0), stop=(c == DC - 1))
                            if stage == 1:
                                nc.scalar.activation(qT[:, oc, :], ps[:, :], AF.Identity,
                                                     bias=qb[:, oc:oc + 1], scale=1.0)
                            else:
                                nc.vector.tensor_scalar_add(qT[:, oc, :], ps[:, :],
                                                            qb[:, oc:oc + 1])
                            ps = psum()
                            for c in range(DC):
                                nc.tensor.matmul(ps[:, :], wk[:, c, 128 * oc:128 * (oc + 1)],
                                                 a1[:, c, :], start=(c == 0), stop=(c == DC - 1))
                            nc.vector.tensor_scalar_add(kT[:, oc, :], ps[:, :], kb[:, oc:oc + 1])
                        # v in [s, d] orientation, zero-padded per head pair:
                        # vzp[:, f, hp, hh, sc2, :]: hh=0 -> [v|0], hh=1 -> [0|v]
                        vzp = apool.tile([128, 2, 6, 2, 2, 128], fp8, tag="vv",
                                         name="vzp", bufs=2)
                        if zero_vzp[0] > 0:
                            zero_vzp[0] -= 1
                            nc.scalar.memzero(
                                vzp[:].rearrange("p a b c d e -> p (a b c d e)"))
                        for sc in range(4):
                            f, sc2 = sc // 2, sc % 2
                            for half in range(2):
                                ps = psum()
                                for c in range(DC):
                                    nc.tensor.matmul(ps[:, 0:384],
                                                     a1[:, c, 128 * sc:128 * (sc + 1)],
                                                     wv[:, c, 384 * half:384 * (half + 1)],
                                                     start=(c == 0), stop=(c == DC - 1))
                                psv = ps[:, 0:384].rearrange(
                                    "p (hp hh d) -> p hp hh d", hp=3, hh=2)
                                for hh in range(2):
                                    nc.vector.tensor_copy(
                                        vzp[:, f, 3 * half:3 * (half + 1), hh, sc2,
                                            64 * hh:64 * hh + 64],
                                        psv[:, :, hh, :])
                        return qT, kT, vzp

                    def attn_att(stage, qkv):
                        qT, kT, vzp = qkv
                        yT = apool.tile([128, DC, 512], bf16, tag="yT", name="yT", bufs=2)
                        for f in range(2):
                            toff = 256 * f
                            for hp in range(6):
                                sps = pp.tile([128, 2, 512], f32, tag="ps2", name="sps", bufs=2)
                                for sc2 in range(2):
                                    for hh in range(2):
                                        nc.tensor.matmul(
                                            sps[:, hh, 256 * sc2:256 * (sc2 + 1)],
                                            kT[64 * hh:64 * (hh + 1), hp,
                                               toff + 128 * sc2:toff + 128 * (sc2 + 1)],
                                            qT[64 * hh:64 * (hh + 1), hp, toff:toff + 256],
                                            start=(sc2 == 0), stop=(sc2 == 1),
                                            tile_position=(64 * hh, 0))
                                if with_mask and stage == 1:
                                    for hh in range(2):
                                        nc.vector.tensor_tensor(
                                            sps[:, hh, :], sps[:, hh, :],
                                            mask_sb[:].rearrange("p s t -> p (s t)"), OP.add)
                                attb = hpool.tile([128, 2, 2, T], fp8, tag="att", name="attb")
                                nc.scalar.activation(
                                    attb[:].rearrange("p h s t -> p (h s t)"),
                                    sps[:].rearrange("p h x -> p (h x)"),
                                    AF.Exp, bias=0.0, scale=1.0 / math.sqrt(HD))
                                att = [attb[:, 0], attb[:, 1]]
                                pd = psum()
                                for hh in range(2):
                                    nc.tensor.matmul(
                                        pd[0:2, 0:T],
                                        cvec8[:, :, 2 * hh:2 * hh + 2],
                                        att[hh], perf_mode=DR,
                                        start=(hh == 0), stop=(hh == 1))
                                yb = psum()
                                for hh in range(2):
                                    nc.tensor.matmul(
                                        yb[:, 0:T],
                                        vzp[:, f, hp, hh],
                                        att[hh], perf_mode=DR,
                                        start=(hh == 0), stop=(hh == 1))
                                pdr = spool.tile([2, 512], bf16, tag="pdr", name="pdr", bufs=4)
                                nc.vector.reciprocal(pdr[0:2, 0:T], pd[0:2, 0:T])
                                nc.tensor.matmul(pd[:, T:2 * T], sel2, pdr[0:2, 0:T],
                                                 start=True, stop=True)
                                r2sb = spool.tile([128, T], bf16, tag="r2sb", name="r2sb", bufs=4)
                                nc.vector.tensor_copy(r2sb[:], pd[:, T:2 * T])
                                nc.vector.tensor_tensor(yT[:, hp, toff:toff + 256],
                                                        yb[:, 0:T], r2sb[:], OP.mult)
                        return yT

                    def attn_oproj(stage, yT, xviews, out_writer):
                        s = stage - 1
                        wo = wt[(s, "o")]
                        cb = bias_col(24 if stage == 1 else 30, DC)
                        for ec in range(DC):
                            ps = psum()
                            for hc in range(DC):
                                nc.tensor.matmul(ps[:, :], wo[:, hc, 128 * ec:128 * (ec + 1)],
                                                 yT[:, hc, :], start=(hc == 0), stop=(hc == DC - 1))
                            out_writer(ec, ps, cb, xviews)

                    # ================= stage 1 (frame order, in-place on x2) =================
                    def x2views(p):
                        return [x2[:, c, 512 * p:512 * (p + 1)] for c in range(DC)]

                    def s1_writer_for(p):
                        def wr(ec, ps, cb, xviews):
                            nc.vector.scalar_tensor_tensor(
                                x2[:, ec, 512 * p:512 * (p + 1)], ps[:, :],
                                cb[:, ec:ec + 1], xviews[ec], OP.add, OP.add)
                        return wr

                    a1s = {}
                    ln_stats(x2views(0), 0, psum)
                    for p in range(9):
                        if p >= 1:
                            qkv = attn_qkv(1, a1s.pop(p - 1))
                        if p < 8:
                            a1 = apool.tile([128, DC, 512], bf16, tag="a1", name="a1", bufs=2)
                            ln_finish(x2[:, :, 512 * p:512 * (p + 1)], a1, p, psum)
                            a1s[p] = a1
                        if p >= 1:
                            yT = attn_att(1, qkv)
                        if p + 1 < 8:
                            ln_stats(x2views(p + 1), p + 1, psum)
                        if p >= 1:
                            attn_oproj(1, yT, x2views(p - 1), s1_writer_for(p - 1))

                    # ================= stage 2 (window order, gather from x2) =================
                    # x2 token idx = f*256 + (4hi+hh)*16 + 4wi+ww ; window w=(hi,wi)
                    x2w = x2[:].rearrange("p c (f hi hh wi ww) -> p c hi wi f hh ww",
                                          f=16, hi=4, hh=4, wi=4)

                    def ln_stats_s2(wp, pidx):
                        """Stage-2 stats direct from x2 (strided views, pre-gather)."""
                        statr, mrz = statrs[pidx % 2], mrzs[pidx % 2]
                        st1, st2_ = psum(), psum()
                        for c in range(DC):
                            for j in range(2):
                                w = 2 * wp + j
                                src_ = x2w[:, c, w // 4, w % 4]   # [128, 16, 4, 4]
                                x2t = spool.tile([128, 256], bf16, tag="x2t2",
                                                 name="x2t2", bufs=3)
                                x2tv = x2t[:].rearrange("p (f hh ww) -> p f hh ww",
                                                        f=16, hh=4)
                                nc.vector.tensor_tensor(x2tv, src_, src_, OP.mult)
                                nc.tensor.matmul(st1[0:1, 256 * j:256 * (j + 1)],
                                                 ones_c, src_,
                                                 start=(c == 0), stop=(c == DC - 1))
                                nc.tensor.matmul(st2_[0:1, 256 * j:256 * (j + 1)],
                                                 ones_c, x2t[:],
                                                 start=(c == 0), stop=(c == DC - 1))
                        t_m = spool.tile([2, 512], f32, tag="vec", name="t_m2", bufs=4)
                        t_e = spool.tile([2, 512], f32, tag="vec", name="t_e2", bufs=4)
                        nc.vector.tensor_scalar_mul(t_m[0:1, :], st1[0:1, :], -1.0 / D)
                        nc.vector.tensor_scalar_mul(t_e[0:1, :], st2_[0:1, :], 1.0 / D)
                        nc.vector.tensor_tensor(mrz[0:1, :], t_m[0:1, :], t_m[0:1, :], OP.mult)
                        nc.vector.tensor_tensor(t_e[0:1, :], t_e[0:1, :], mrz[0:1, :], OP.subtract)
                        nc.vector.tensor_scalar_add(t_e[0:1, :], t_e[0:1, :], EPS)
                        nc.scalar.activation(t_e[0:1, :], t_e[0:1, :], AF.Ln,
                                             bias=0.0, scale=1.0)
                        nc.scalar.activation(statr[0:1, :], t_e[0:1, :], AF.Exp,
                                             bias=0.0, scale=-0.5)
                        nc.vector.tensor_tensor(mrz[0:1, :], t_m[0:1, :],
                                                statr[0:1, :], OP.mult)

                    def gather(wp):
                        xs2 = opool.tile([128, DC, 512], bf16, tag="xs", name="xs2", bufs=2)
                        for c in range(DC):
                            for j in range(2):
                                w = 2 * wp + j
                                src = x2w[:, c, w // 4, w % 4]  # [128, 16, 4, 4]
                                dst = xs2[:, c, 256 * j:256 * (j + 1)]
                                nc.scalar.copy(
                                    dst.rearrange("p (f hh ww) -> p f hh ww", f=16, hh=4),
                                    src)
                        return xs2

                    def s2_writer_for(oo2):
                        def wr(ec, ps, cb, xviews):
                            nc.vector.scalar_tensor_tensor(
                                oo2[:, ec, :], ps[:, :], cb[:, ec:ec + 1],
                                xviews[ec], OP.add, OP.add)
                        return wr

                    prev = None
                    ln_stats_s2(0, 0)
                    xs2n = gather(0)
                    for wp in range(9):
                        xs2 = xs2n
                        if wp >= 1:
                            qkv = attn_qkv(2, prev[1])
                        if wp < 8:
                            a1 = apool.tile([128, DC, 512], bf16, tag="a1", name="a1b", bufs=2)
                            ln_finish(xs2[:], a1, wp, psum)
                        if wp >= 1:
                            yT = attn_att(2, qkv)
                        if wp + 1 < 8:
                            ln_stats_s2(wp + 1, wp + 1)
                            xs2n = gather(wp + 1)
                        if wp >= 1:
                            p_ = wp - 1
                            oo2 = opool.tile([128, DC, 512], bf16, tag="oo2", name="oo2", bufs=2)
                            attn_oproj(2, yT, [prev[0][:, c, :] for c in range(DC)],
                                       s2_writer_for(oo2))
                            nc.sync.dma_start(m2.ap()[p_], oo2[:])
                        if wp < 8:
                            prev = (xs2, a1)

                # ================= MLP (window order slabs from m2) =================
                st3 = contextlib.ExitStack()
                with st3:
                    wmpool = st3.enter_context(tc.tile_pool(name="wtsm", bufs=2))
                    mpool = st3.enter_context(tc.tile_pool(name="mact", bufs=2))
                    gpool = st3.enter_context(tc.tile_pool(name="gm", bufs=1))
                    ppB = st3.enter_context(tc.tile_pool(name="psB", bufs=8, space="PSUM"))

                    def psumB():
                        return ppB.tile([128, 512], f32, tag="psb", name="psb")

                    w1 = wmpool.tile([128, DC, FH], bf16, tag="wm", name="w1")
                    nc.gpsimd.dma_start(w1[:], w1d.ap())
                    w2 = wmpool.tile([128, FHC, D], bf16, tag="wm", name="w2")
                    nc.gpsimd.dma_start(w2[:], w2d.ap())
                    f1b = bias_col(42, FHC)
                    f2b = bias_col(36, DC)

                    def ln512B(xviews, a_out, pidx):
                        statr, mrz = statrs[pidx % 2], mrzs[pidx % 2]
                        st1, st2_ = psumB(), psumB()
                        for c in range(DC):
                            x2t = spool.tile([128, 512], bf16, tag="x2t", name="x2tB", bufs=3)
                            nc.scalar.activation(x2t[:], xviews[c], AF.Square,
                                                 bias=0.0, scale=1.0)
                            nc.tensor.matmul(st1[0:1, :], ones_c, xviews[c],
                                             start=(c == 0), stop=(c == DC - 1))
                            nc.tensor.matmul(st2_[0:1, :], ones_c, x2t[:],
                                             start=(c == 0), stop=(c == DC - 1))
                        t_m = spool.tile([2, 512], f32, tag="vec", name="t_mB", bufs=4)
                        t_e = spool.tile([2, 512], f32, tag="vec", name="t_eB", bufs=4)
                        nc.vector.tensor_scalar_mul(t_m[0:1, :], st1[0:1, :], -1.0 / D)
                        nc.vector.tensor_scalar_mul(t_e[0:1, :], st2_[0:1, :], 1.0 / D)
                        nc.vector.tensor_tensor(mrz[0:1, :], t_m[0:1, :], t_m[0:1, :], OP.mult)
                        nc.vector.tensor_tensor(t_e[0:1, :], t_e[0:1, :], mrz[0:1, :], OP.subtract)
                        nc.vector.tensor_scalar_add(t_e[0:1, :], t_e[0:1, :], EPS)
                        nc.scalar.activation(t_e[0:1, :], t_e[0:1, :], AF.Sqrt,
                                             bias=0.0, scale=1.0)
                        nc.vector.reciprocal(statr[0:1, :], t_e[0:1, :])
                        nc.vector.tensor_tensor(mrz[0:1, :], t_m[0:1, :],
                                                statr[0:1, :], OP.mult)
                        rps, mps = psumB(), psumB()
                        nc.tensor.matmul(rps[:, :], bcast1, statr[:, :], start=True, stop=True)
                        nc.tensor.matmul(mps[:, :], bcast1, mrz[:, :], start=True, stop=True)
                        rsb = spool.tile([128, 512], bf16, tag="rmsb", name="rsbB", bufs=4)
                        msb = spool.tile([128, 512], bf16, tag="rmsb", name="msbB", bufs=4)
                        nc.scalar.copy(rsb[:], rps[:, :])
                        nc.scalar.copy(msb[:], mps[:, :])
                        for c in range(DC):
                            tmp = spool.tile([128, 512], bf16, tag="x2t", name="lntmpB", bufs=3)
                            nc.vector.tensor_tensor(tmp[:], xviews[c], rsb[:], OP.mult)
                            nc.vector.tensor_tensor(a_out[:, c, :], tmp[:], msb[:], OP.add)

                    def mlp_fc1(h3):
                        g1 = gpool.tile([128, FHC, 512], bf16, tag="g1", name="g1", bufs=2)
                        for oc in range(FHC):
                            ps1 = psumB()
                            for c in range(DC):
                                nc.tensor.matmul(ps1[:, :], w1[:, c, 128 * oc:128 * (oc + 1)],
                                                 h3[:, c, :], start=(c == 0), stop=(c == DC - 1))
                            nc.scalar.activation(g1[:, oc, :], ps1[:, :], AF.Gelu,
                                                 bias=f1b[:, oc:oc + 1], scale=1.0)
                        return g1

                    def mlp_fc2(xs3, g1, sl):
                        oo = mpool.tile([128, DC, 512], f32, tag="oo", name="oo", bufs=2)
                        for ec in range(DC):
                            zps = psumB()
                            for oc in range(FHC):
                                nc.tensor.matmul(zps[:, :], w2[:, oc, 128 * ec:128 * (ec + 1)],
                                                 g1[:, oc, :], start=(oc == 0), stop=(oc == FHC - 1))
                            nc.vector.scalar_tensor_tensor(
                                oo[:, ec, :], zps[:, :], f2b[:, ec:ec + 1],
                                xs3[:, ec, :], OP.add, OP.add)
                        nc.sync.dma_start(ot.ap()[sl], oo[:])

                    prevm = None
                    for sl in range(9):
                        if sl >= 1:
                            g1p = mlp_fc1(prevm[1])
                        if sl < 8:
                            xs3 = mpool.tile([128, DC, 512], bf16, tag="xs3", name="xs3", bufs=2)
                            nc.sync.dma_start(xs3[:], m2.ap()[sl])
                            ln_stats([xs3[:, c, :] for c in range(DC)], sl, psumB)
                        if sl >= 1:
                            mlp_fc2(prevm[0], g1p, sl - 1)
                        if sl < 8:
                            h3 = mpool.tile([128, DC, 512], bf16, tag="h3", name="h3", bufs=2)
                            ln_finish(xs3[:], h3, sl, psumB)
                            prevm = (xs3, h3)

            if iters == 1:
                body()
            else:
                with tc.For_i(0, iters, 1) as iv:
                    body(iv)

    nc.compile()
    return nc


def _host_prep(inputs):
    """Build per-core input maps (bf16 weights with LN folding)."""
    import ml_dtypes
    f32 = np.float32
    bfl = ml_dtypes.bfloat16
    x = np.asarray(inputs["x"], f32)
    mask = np.asarray(inputs["mask"])
    with_mask = not bool((mask == 1).all())

    g = {i: np.asarray(inputs[f"ln{i}_g"], f32) for i in (1, 2, 3)}
    b = {i: np.asarray(inputs[f"ln{i}_b"], f32) for i in (1, 2, 3)}

    def wmat(key):
        return np.asarray(inputs[key], f32)

    def pack_w(w):
        # [in, out] -> [128, kc, out]
        kc = w.shape[0] // 128
        return np.ascontiguousarray(
            w.reshape(kc, 128, w.shape[1]).transpose(1, 0, 2)).astype(bfl)

    com = {}
    wa = np.zeros((2, 128, 4, DC, D), bfl)
    for s in (1, 2):
        gs, bs = g[s], b[s]
        for mi, mn in enumerate(("q", "k", "v", "o")):
            w = wmat(f"{mn}{s}_w")
            if mn != "o":
                w = gs[:, None] * w
            wa[s - 1, :, mi] = pack_w(w).reshape(128, DC, D)
    com["wad"] = wa
    com["w1d"] = pack_w(g[3][:, None] * wmat("fc1_w")).reshape(128, DC, FH)
    com["w2d"] = pack_w(wmat("fc2_w")).reshape(128, FHC, D)

    def chunks(vec, w):
        return np.asarray(vec, f32).reshape(w, 128).T

    biasp = np.zeros((128, 66), f32)
    qb1 = b[1] @ wmat("q1_w") + np.asarray(inputs["q1_b"], f32)
    kb1 = b[1] @ wmat("k1_w") + np.asarray(inputs["k1_b"], f32)
    qb2 = b[2] @ wmat("q2_w") + np.asarray(inputs["q2_b"], f32)
    kb2 = b[2] @ wmat("k2_w") + np.asarray(inputs["k2_b"], f32)
    vb1 = b[1] @ wmat("v1_w") + np.asarray(inputs["v1_b"], f32)
    vb2 = b[2] @ wmat("v2_w") + np.asarray(inputs["v2_b"], f32)
    c1 = vb1 @ wmat("o1_w") + np.asarray(inputs["o1_b"], f32)
    c2 = vb2 @ wmat("o2_w") + np.asarray(inputs["o2_b"], f32)
    f1b = b[3] @ wmat("fc1_w") + np.asarray(inputs["fc1_b"], f32)
    biasp[:, 0:6] = chunks(qb1, DC)
    biasp[:, 6:12] = chunks(kb1, DC)
    biasp[:, 12:18] = chunks(qb2, DC)
    biasp[:, 18:24] = chunks(kb2, DC)
    biasp[:, 24:30] = chunks(c1, DC)
    biasp[:, 30:36] = chunks(c2, DC)
    biasp[:, 36:42] = chunks(np.asarray(inputs["fc2_b"], f32), DC)
    biasp[:, 42:66] = chunks(f1b, FHC)
    com["biasd"] = biasp

    cvecd = np.zeros((128, 8), bfl)
    cvecd[:, 0] = 1
    cvecd[:, 1] = 1
    cvecd[:, 4] = 1
    com["cvecd"] = cvecd
    cvec8d = np.zeros((128, 2, 16), ml_dtypes.float8_e4m3fn)
    cvec8d[:, :, 0] = 1    # hh0: m0 <- ones
    cvec8d[:, :, 3] = 1    # hh1: m1 <- ones
    com["cvec8d"] = cvec8d

    rowd = np.zeros((2, 768), bfl)
    rowd[0, 0:128] = 1                      # bcast1 row0
    rowd[0, 128:192] = 1                    # sel2 row0 -> partitions 0:64
    rowd[1, 192:256] = 1                    # sel2 row1 -> partitions 64:128
    com["rowd"] = rowd

    if with_mask:
        madd = (mask.reshape(T, T).astype(f32) - 1.0) * 1e9
        com["maskd"] = np.ascontiguousarray(
            madd.T.reshape(2, 128, T).transpose(1, 0, 2))

    in_maps = []
    for c in range(NCORE):
        shard = x[NF * c:NF * (c + 1)]                  # [16, 256, 768]
        xT = shard.reshape(NT, D).T                     # [768, 4096]
        m = dict(com)
        m["xt"] = np.ascontiguousarray(
            xT.reshape(DC, 128, 8, 512).transpose(2, 1, 0, 3)).astype(bfl)
        in_maps.append(m)
    return in_maps, with_mask


def _host_post(results, dtype):
    """results: list of 8 dicts with 'ot' [8, 128, DC, 512] f32, window order."""
    out = np.empty((NCORE * NF, T, D), dtype)
    for c, r in enumerate(results):
        o = np.asarray(r["ot"], np.float32)             # [8, 128, 6, 512]
        o = o.transpose(2, 1, 0, 3).reshape(D, NT)      # [768, 4096] wtoken order
        # wtoken = (hi, wi, f, hh, ww)
        o = o.reshape(D, 4, 4, NF, 4, 4)
        o = o.transpose(3, 1, 4, 2, 5, 0)               # (f, hi, hh, wi, ww, d)
        out[NF * c:NF * (c + 1)] = o.reshape(NF, T, D)
    return out


def kernel(**inputs) -> np.ndarray:
    from concourse.bass_utils import run_bass_kernel_spmd

    in_maps, with_mask = _host_prep(inputs)
    key = ("k", with_mask)
    if key not in _CACHE:
        _CACHE[key] = _build(with_mask)
    nc = _CACHE[key]
    res = run_bass_kernel_spmd(nc, in_maps, core_ids=list(range(NCORE)))
    return _host_post(res.results, np.asarray(inputs["x"]).dtype)
